# revision 23
# baseline (speedup 1.0000x reference)
# Trainium2 Bass kernel for nn_CvtLstm: ConvLSTM cell with 4-branch,
# 4-head spatial attention. Data-parallel over batch N=32 across 8
# NeuronCores (4 samples per core); weights replicated to every core.
#
# Per-core layout: channels on partitions, flattened 16x16 spatial (256)
# on the free dim. conv3x3 = 9 shifted matmuls reading a zero-padded
# [128, 2, 18, 18] tile. Attention scores are computed directly in the
# transposed [d, q] layout (lhsT = per-head k rows, K=32 row-partial
# matmuls); exp on the ACT engine with no max subtraction (scores lie in
# [-9, 8]); the PV product and the softmax denominator Z come from one
# M=64 matmul per (head, d-chunk) whose weight columns are [vT_g | ones].
#
# Engine budget decisions (from TimelineSim trace analysis):
# - No per-iter DMAs: the old head-restack DMAs serialized on the single
#   HWDGE queue (625ns/descriptor). Normalization is now: two PSUM->SBUF
#   extraction copies (a-rows on DVE, z-rows alternating DVE/ACT) plus 4
#   partition-scattering divides on the otherwise-idle Pool engine
#   (GPSIMD cannot read PSUM; both-SBUF operands must share a base
#   partition, hence both extracts land at base 0).
# - All gates use Tanh (sigmoid(x) = (1+tanh(x/2))/2 folded into scales/
#   weights) so the single act table set {exp, tanh, copy} is loaded
#   once; sigmoid and exp never coexist in one table set.
# - LSTM update is 4 fused scalar_tensor_tensor ops: (t+1)*x computes
#   2*gate*x without materializing the 0.5s, which are folded into
#   tanh-scale and W_out.
# - Zero-padding and the vT ones-columns are Pool memsets, not DMAs.
# - Weights ship in 4 packed DMAs ordered by first use (winT -> conv ->
#   q/k/v -> gates/out) so the PE pipeline starts ~1.5us in.
#
# Hardware constraint honored throughout: two row-partial matmuls at
# different row groups back-to-back fault the device; full-row dummy
# separators are inserted at group changes (see baseline notes).

import numpy as np

N, I, H, W = 32, 64, 16, 16
R, CM, A, HEADS, HC = 128, 128, 128, 4, 32
HW = H * W           # 256
S = 4                # samples per core
NCORES = 8

_CACHE = {}

# packed weight column offsets (f32 words per partition)
W0_WIN, W0_BIN, W0_ONES, W0_END = 0, 128, 129, 161
W1_CONV, W1_END = 0, 2304
W2_QK, W2_V, W2_END = 0, 1024, 1536
W3_TOK, W3_SKIP, W3_OUT, W3_BTOK, W3_BOUT, W3_END = 0, 2048, 3072, 3200, 3204, 3205


def _build_program():
    import contextlib
    import concourse.bacc as bacc
    import concourse.mybir as mybir
    import concourse.tile as tile

    F32 = mybir.dt.float32
    F32R = mybir.dt.float32r
    BF16 = mybir.dt.bfloat16
    AF = mybir.ActivationFunctionType
    ALU = mybir.AluOpType

    nc = bacc.Bacc("TRN2", target_bir_lowering=False, debug=False)

    def dram(name, shape, kind="ExternalInput"):
        return nc.dram_tensor(name, list(shape), F32, kind=kind).ap()

    xin = dram("xin", [S, I, HW])
    hin = dram("hin", [S, R, HW])
    cin = dram("cin", [S, R, HW])
    w0d = dram("w0", [128, W0_END])
    w1d = dram("w1", [128, W1_END])
    w2d = dram("w2", [128, W2_END])
    w3d = dram("w3", [128, W3_END])
    yout = dram("yout", [S, R, HW], kind="ExternalOutput")

    QSRC = [0, 0, 1, 1]   # q source per branch: 0=xc, 1=hc
    KSRC = [0, 1, 0, 1]   # k/v source per branch
    BORDER = [3, 1, 2, 0]  # per-pass branch order (b3 = pure hc, earliest)
    GSCALE = [0.5, 0.5, 1.0, 0.5]  # tanh input scale per gate (i,f,g,o)

    with tile.TileContext(nc) as tc:
        with contextlib.ExitStack() as ctx:
            wpool = ctx.enter_context(tc.tile_pool(name="wts", bufs=1))
            sbA = ctx.enter_context(tc.tile_pool(name="sbA", bufs=2))
            sbB = ctx.enter_context(tc.tile_pool(name="sbB", bufs=2))
            epool = ctx.enter_context(tc.tile_pool(name="ep", bufs=2))
            stp = ctx.enter_context(tc.tile_pool(name="st", bufs=2, space="PSUM"))
            azp = ctx.enter_context(tc.tile_pool(name="az", bufs=2, space="PSUM"))
            pwp = ctx.enter_context(tc.tile_pool(name="pw", bufs=2, space="PSUM"))

            # ---------------- input DMAs for pass 0 (issued first so the
            # XT matmul can start while the big weight DMAs stream) -----
            h_pad0 = sbA.tile([128, 648], F32R, tag="hpad", name="hpad")
            xt_pad0 = sbA.tile([128, 648], F32R, tag="xtpad", name="xtpad")
            x2_0 = sbA.tile([64, 2, 256], F32R, tag="x2", name="x2")
            nc.gpsimd.memset(h_pad0.bitcast(F32), 0.0)
            nc.gpsimd.memset(xt_pad0.bitcast(F32), 0.0)
            hv0 = h_pad0.rearrange("p (s y x) -> p s y x", s=2, y=18, x=18)
            for s in range(2):
                nc.sync.dma_start(
                    out=hv0[:, s, 1:17, 1:17],
                    in_=hin[s].rearrange("c (h w) -> c h w", h=16).bitcast(F32R))
            nc.sync.dma_start(
                out=x2_0, in_=xin[0:2].rearrange("s c q -> c s q").bitcast(F32R))

            # ---------------- weights to SBUF (4 packed DMAs) ----------
            def wload(name, src, w):
                t = wpool.tile([128, w], F32R, tag=name, name=name)
                nc.sync.dma_start(out=t, in_=src.bitcast(F32R))
                return t

            w0_s = wload("w0", w0d, W0_END)
            w1_s = wload("w1", w1d, W1_END)
            w2_s = wload("w2", w2d, W2_END)
            w3_s = wload("w3", w3d, W3_END)

            winT_s = w0_s[0:64, W0_WIN:W0_WIN + 128]
            b_in_s = w0_s[:, W0_BIN:W0_BIN + 1].bitcast(F32)
            ones_s = w0_s[:, W0_ONES:W0_ONES + 32]
            wconvT_s = w1_s[:, W1_CONV:W1_CONV + 2304].rearrange(
                "p (s t o) -> p s t o", s=2, t=9)
            wqkT_s = w2_s[:, W2_QK:W2_QK + 1024].rearrange(
                "p (k b o) -> p k b o", k=2, b=4)
            wvT_s = w2_s[:, W2_V:W2_V + 512].rearrange("p (s q) -> p s q", s=2)
            wtokT_s = w3_s[:, W3_TOK:W3_TOK + 2048].rearrange(
                "p (g b o) -> p g b o", g=4, b=4)
            wskipT_s = w3_s[:, W3_SKIP:W3_SKIP + 1024].rearrange(
                "p (g s o) -> p g s o", g=4, s=2)
            woutT_s = w3_s[:, W3_OUT:W3_OUT + 128]
            btok_s = w3_s[:, W3_BTOK:W3_BTOK + 4].bitcast(F32)
            bout_s = w3_s[:, W3_BOUT:W3_BOUT + 1].bitcast(F32)

            # ---------------- per-pass state ----------------
            xc_sb = [None, None]
            hc_sb = [None, None]
            q_sb = [[None] * 4, [None] * 4]
            k_sb = [[None] * 4, [None] * 4]
            vt_sb = [[None] * 4, [None] * 4]
            a_all = [None, None]
            cprev_sb = [None, None]
            gate_sb = [[None] * 4, [None] * 4]

            def emit_input_pads(p):
                """zero-padded xt/h tiles + x2 + XT matmul + tanh.
                For p=0 the pads/DMAs were already emitted above."""
                if p == 0:
                    xt_pad, h_pad, x2 = xt_pad0, h_pad0, x2_0
                else:
                    h_pad = sbA.tile([128, 648], F32R, tag="hpad", name="hpad")
                    xt_pad = sbA.tile([128, 648], F32R, tag="xtpad", name="xtpad")
                    x2 = sbA.tile([64, 2, 256], F32R, tag="x2", name="x2")
                    nc.gpsimd.memset(h_pad.bitcast(F32), 0.0)
                    nc.gpsimd.memset(xt_pad.bitcast(F32), 0.0)
                    hv = h_pad.rearrange("p (s y x) -> p s y x", s=2, y=18, x=18)
                    for s in range(2):
                        nc.sync.dma_start(
                            out=hv[:, s, 1:17, 1:17],
                            in_=hin[2 * p + s].rearrange(
                                "c (h w) -> c h w", h=16).bitcast(F32R))
                    nc.sync.dma_start(
                        out=x2,
                        in_=xin[2 * p:2 * p + 2].rearrange(
                            "s c q -> c s q").bitcast(F32R))
                XT = pwp.tile([128, 512], F32, tag="pw", name="XT")
                nc.tensor.matmul(out=XT, lhsT=winT_s,
                                 rhs=x2.rearrange("p s q -> p (s q)"),
                                 start=True, stop=True)
                xv = xt_pad.rearrange("p (s y x) -> p s y x", s=2, y=18, x=18)
                nc.scalar.activation(
                    out=xv[:, :, 1:17, 1:17],
                    in_=XT.rearrange("p (s h w) -> p s h w", s=2, h=16, w=16),
                    func=AF.Tanh, bias=b_in_s)
                return xt_pad, h_pad

            def emit_conv(p, src, pad):
                """3x3 SAME conv via 9 shifted matmuls; src 0=xc, 1=hc."""
                CP = pwp.tile([128, 512], F32, tag="pw", name="CP")
                pv = pad.rearrange("p (s y x) -> p s y x", s=2, y=18, x=18)
                for t in range(9):
                    ky, kx = divmod(t, 3)
                    nc.tensor.matmul(out=CP, lhsT=wconvT_s[:, src, t, :],
                                     rhs=pv[:, :, ky:ky + 16, kx:kx + 16],
                                     start=(t == 0), stop=(t == 8))
                dst = sbA.tile([128, 512], F32R, tag=("xc" if src == 0 else "hc"),
                               name=("xc" if src == 0 else "hc"))
                nc.vector.tensor_copy(dst, CP)
                if src == 0:
                    xc_sb[p] = dst
                else:
                    hc_sb[p] = dst

            def emit_qk(p, b):
                srcq = xc_sb[p] if QSRC[b] == 0 else hc_sb[p]
                srck = xc_sb[p] if KSRC[b] == 0 else hc_sb[p]
                QB = pwp.tile([128, 512], F32, tag="pw", name="QB")
                nc.tensor.matmul(out=QB, lhsT=wqkT_s[:, 0, b, :], rhs=srcq,
                                 start=True, stop=True)
                q_sb[p][b] = sbB.tile([128, 512], BF16, tag=f"q{b}", name=f"q{b}")
                nc.vector.tensor_copy(q_sb[p][b], QB)
                KB = pwp.tile([128, 512], F32, tag="pw", name="KB")
                nc.tensor.matmul(out=KB, lhsT=wqkT_s[:, 1, b, :], rhs=srck,
                                 start=True, stop=True)
                k_sb[p][b] = sbB.tile([128, 512], BF16, tag=f"k{b}", name=f"k{b}")
                nc.vector.tensor_copy(k_sb[p][b], KB)

            def emit_vt(p, src):
                """vT for the two branches fed by `src`; fills [vT_g | ones]
                64-wide head blocks of vt_sb[b] = [128, (s c) 4, 256]."""
                b0 = src            # branches (0,2) from xc, (1,3) from hc
                for b in (b0, b0 + 2):
                    if vt_sb[p][b] is None:
                        vt_sb[p][b] = sbB.tile([128, 1024], BF16,
                                               tag=f"vt{b}", name=f"vt{b}")
                        ov = vt_sb[p][b].rearrange(
                            "p (n d) -> p n d", n=16)[:, :, 32:64]
                        nc.gpsimd.memset(ov, 1.0)
                src_sb = xc_sb[p] if src == 0 else hc_sb[p]
                sv = src_sb.rearrange("p (s c d) -> p s c d", s=2, c=2)
                for s in range(2):
                    for c in range(2):
                        VT = pwp.tile([128, 256], F32, tag="pw", name="VT")
                        nc.tensor.matmul(out=VT, lhsT=sv[:, s, c, :],
                                         rhs=wvT_s[:, src, :],
                                         start=True, stop=True)
                        sc = s * 2 + c
                        for j, b in enumerate((b0, b0 + 2)):
                            dst = vt_sb[p][b][:, sc * 256:(sc + 1) * 256]
                            dst = dst.rearrange("p (g d) -> p g d", g=4)[:, :, 0:32]
                            srcv = VT[:, j * 128:(j + 1) * 128].rearrange(
                                "p (g d) -> p g d", g=4)
                            nc.vector.tensor_copy(dst, srcv)

            def emit_cprev(p):
                cprev_sb[p] = sbA.tile([128, 512], F32, tag="cprev", name="cprev")
                nc.sync.dma_start(
                    out=cprev_sb[p],
                    in_=cin[2 * p:2 * p + 2].rearrange("s c q -> c s q"))

            # ---------------- attention iteration pieces ----------------
            def emit_scores_exp(p, b, s):
                """returns pT tile [128, 2048] bf16 = exp(scores^T), layout
                (g, c, q) 4x2x256. One ST bank + one exp per head so PV(g)
                can start as soon as exp(g) lands."""
                kv = k_sb[p][b].rearrange("p (s c d) -> p s c d", s=2, c=2)
                qv = q_sb[p][b].rearrange("p (s q) -> p s q", s=2)
                pT = sbB.tile([128, 2048], BF16, tag="pt", name="pT")
                for h in range(2):
                    ST = stp.tile([128, 1024], F32, tag="st", name="ST")
                    for gg in range(2):
                        g = 2 * h + gg
                        if (h, gg) != (0, 0):
                            # full-row dummy separator into a slice the next
                            # score matmul overwrites (start=True clears it)
                            dsl = ST[0:32, 512:544] if gg == 1 else ST[0:32, 0:32]
                            nc.tensor.matmul(out=dsl, lhsT=ones_s,
                                             rhs=ones_s, start=True, stop=True,
                                             skip_group_check=True)
                        for c in range(2):
                            nc.tensor.matmul(
                                out=ST[:, gg * 512 + c * 256:gg * 512 + c * 256 + 256],
                                lhsT=kv[32 * g:32 * g + 32, s, c, :],
                                rhs=qv[32 * g:32 * g + 32, s, :],
                                start=True, stop=True, skip_group_check=True,
                                tile_position=(32 * g, 0))
                    nc.scalar.activation(out=pT[:, h * 1024:(h + 1) * 1024],
                                         in_=ST, func=AF.Exp)
                return pT

            def emit_pvz(p, b, s, pT):
                # AZ packs 4 heads into one PSUM bank [128, 512]: head g at
                # partitions (g%2)*64 (v rows then z rows), window (g//2)*256.
                AZ = azp.tile([128, 512], F32, tag="az", name="AZ")
                for g in range(4):
                    po, wo = (g % 2) * 64, (g // 2) * 256
                    for c in range(2):
                        sc = s * 2 + c
                        nc.tensor.matmul(
                            out=AZ[po:po + 64, wo:wo + 256],
                            lhsT=vt_sb[p][b][:, sc * 256 + 64 * g:sc * 256 + 64 * g + 64],
                            rhs=pT[:, g * 512 + c * 256:g * 512 + c * 256 + 256],
                            start=(c == 0), stop=(c == 1), skip_group_check=True)
                return AZ

            def emit_norm(p, b, s, AZ, k):
                """a_all[32g:32g+32, slot] = AZ.a[g] * (1/AZ.z[g]). There is
                no HW divide: 1/z via the DVE reciprocal ISA op straight from
                PSUM, a-rows extracted base-0 (alternating DVE/ACT), then 4
                partition-scattering multiplies on the Pool engine (SBUF-only
                engine, mult supported, equal base partitions)."""
                slot = b * 2 + s
                zc = epool.tile([32, 1024], F32, tag="zc", name="zc")
                af = epool.tile([32, 1024], F32, tag="afa", name="af")
                # recip's custom-DVE uop chain mis-reads PSUM: z must be
                # staged through SBUF. AZ rows: 0:32 a(g0,g2), 32:64 z(g0,g2),
                # 64:96 a(g1,g3), 96:128 z(g1,g3); windows (g//2)*256.
                # One [32,512] extraction per engine per tensor: DVE takes the
                # even rows, ACT the odd rows, so neither engine eats the
                # whole 2048-free extraction.
                nc.vector.tensor_copy(af[:, 0:512], AZ[0:32, :])
                nc.scalar.activation(out=af[:, 512:1024], in_=AZ[64:96, :],
                                     func=AF.Copy)
                nc.vector.tensor_copy(zc[:, 0:512], AZ[32:64, :])
                nc.scalar.activation(out=zc[:, 512:1024], in_=AZ[96:128, :],
                                     func=AF.Copy)
                rz = epool.tile([32, 1024], F32, tag="rz", name="rz")
                nc.vector.reciprocal_approx_fast(out=rz, in_=zc)
                if a_all[p] is None:
                    a_all[p] = sbA.tile([128, 2048], F32R, tag="aall", name="a_all")
                # af/rz column order is (g0, g2, g1, g3)
                av = af.rearrange("p (g q) -> p g q", g=4)
                rv = rz.rearrange("p (g q) -> p g q", g=4)
                for g in range(4):
                    col = (g % 2) * 2 + (g // 2)
                    nc.gpsimd.tensor_tensor(
                        out=a_all[p][32 * g:32 * g + 32,
                                     slot * 256:(slot + 1) * 256],
                        in0=av[:, col, :], in1=rv[:, col, :], op=ALU.mult)

            # ---------------- gates / state / output ----------------
            def emit_gate(p, gi):
                G = pwp.tile([128, 512], F32, tag="pw", name="G")
                av = a_all[p].rearrange("p (b s q) -> p b (s q)", b=4, s=2)
                for b in range(4):
                    nc.tensor.matmul(out=G, lhsT=wtokT_s[:, gi, b, :],
                                     rhs=av[:, b, :],
                                     start=(b == 0), stop=False)
                nc.tensor.matmul(out=G, lhsT=wskipT_s[:, gi, 0, :],
                                 rhs=xc_sb[p], start=False, stop=False)
                nc.tensor.matmul(out=G, lhsT=wskipT_s[:, gi, 1, :],
                                 rhs=hc_sb[p], start=False, stop=True)
                gate_sb[p][gi] = sbA.tile([128, 512], F32, tag=f"gate{gi}",
                                          name=f"gate{gi}")
                # all gates via Tanh: sigmoid(y) = (1+tanh(y/2))/2; the /2 is
                # in GSCALE + host-halved biases, the (1+t)/2 in the update.
                nc.scalar.activation(out=gate_sb[p][gi], in_=G, func=AF.Tanh,
                                     scale=GSCALE[gi], bias=btok_s[:, gi:gi + 1])

            def emit_update_out(p):
                ti, tf, tg, to = gate_sb[p]
                # c2 = 2c = (1+tf)*c_prev + (1+ti)*g  (stt is DVE-only;
                # the plain add runs on Pool, which only has tensor_tensor)
                s1 = sbA.tile([128, 512], F32, tag="fc", name="s1")
                nc.vector.scalar_tensor_tensor(
                    out=s1, in0=tf, scalar=1.0, in1=cprev_sb[p],
                    op0=ALU.add, op1=ALU.mult)
                s2 = sbA.tile([128, 512], F32, tag="ig", name="s2")
                nc.vector.scalar_tensor_tensor(
                    out=s2, in0=ti, scalar=1.0, in1=tg,
                    op0=ALU.add, op1=ALU.mult)
                c2 = sbA.tile([128, 512], F32, tag="c", name="c2")
                nc.gpsimd.tensor_add(c2, s1, s2)
                # tanh(c) = tanh(c2/2); h2 = 2h = (1+to)*tanh(c); W_out is
                # pre-halved on the host to absorb the remaining 1/2.
                tcs = sbA.tile([128, 512], F32, tag="tc", name="tcs")
                nc.scalar.activation(out=tcs, in_=c2, func=AF.Tanh, scale=0.5)
                hs = sbA.tile([128, 512], F32R, tag="h", name="hs")
                nc.vector.scalar_tensor_tensor(
                    out=hs, in0=to, scalar=1.0, in1=tcs,
                    op0=ALU.add, op1=ALU.mult)
                OUT = pwp.tile([128, 512], F32, tag="pw", name="OUT")
                nc.tensor.matmul(out=OUT, lhsT=woutT_s, rhs=hs,
                                 start=True, stop=True)
                osb = sbA.tile([128, 512], F32, tag="out", name="osb")
                nc.scalar.activation(out=osb, in_=OUT, func=AF.Identity,
                                     bias=bout_s[:, 0:1])
                nc.sync.dma_start(
                    out=yout[2 * p:2 * p + 2].rearrange("s c q -> c s q"),
                    in_=osb.rearrange("p (s q) -> p s q", s=2))

            # ---------------- emission schedule ----------------
            # prologue: pass-0 essentials up to branch 3 (pure hc)
            xt_pad0r, h_pad0r = emit_input_pads(0)
            emit_conv(0, 1, h_pad0r)     # hc pass0
            emit_qk(0, 3)
            emit_vt(0, 1)                # vT for b1, b3 (hc source)
            pads1 = [None]

            def filler(i):
                if i == 0:
                    emit_conv(0, 0, xt_pad0r)         # xc pass0
                elif i == 1:
                    emit_qk(0, 1)
                    emit_qk(0, 2)
                elif i == 2:
                    emit_qk(0, 0)
                    emit_vt(0, 0)
                    emit_cprev(0)
                elif i == 3:
                    pads1[0] = emit_input_pads(1)
                elif i == 4:
                    emit_conv(1, 1, pads1[0][1])      # hc pass1
                elif i == 5:
                    emit_conv(1, 0, pads1[0][0])      # xc pass1
                elif i == 6:
                    emit_qk(1, 3)
                    emit_vt(1, 1)
                elif i == 7:
                    emit_qk(1, 1)
                    emit_qk(1, 2)
                elif i == 8:
                    emit_qk(1, 0)
                    emit_vt(1, 0)
                    emit_cprev(1)
                elif i in (9, 10, 11, 12):
                    emit_gate(0, i - 9)
                elif i == 13:
                    emit_update_out(0)

            iters = [(p, b, s) for p in (0, 1) for b in BORDER for s in (0, 1)]
            prev = None
            for i, (p, b, s) in enumerate(iters):
                pT = emit_scores_exp(p, b, s)
                if prev is not None:
                    pp, pb, ps, ppT = prev
                    AZ = emit_pvz(pp, pb, ps, ppT)
                    emit_norm(pp, pb, ps, AZ, i)
                prev = (p, b, s, pT)
                filler(i)
            pp, pb, ps, ppT = prev
            AZ = emit_pvz(pp, pb, ps, ppT)
            emit_norm(pp, pb, ps, AZ, len(iters))
            for gi in range(4):
                emit_gate(1, gi)
            emit_update_out(1)

    nc.compile()
    return nc


def _prep_shared(inputs):
    f = np.float32
    c = np.ascontiguousarray
    W_cx, W_ch = np.asarray(inputs["W_cx"], f), np.asarray(inputs["W_ch"], f)
    W_q, W_k, W_v = (np.asarray(inputs[k], f) for k in ("W_q", "W_k", "W_v"))
    W_tok, W_skip = np.asarray(inputs["W_tok"], f), np.asarray(inputs["W_skip"], f)

    w0 = np.zeros((128, W0_END), f)
    w0[0:64, W0_WIN:W0_WIN + 128] = np.asarray(inputs["W_in"], f).T
    w0[:, W0_BIN] = np.asarray(inputs["b_in"], f)
    w0[:, W0_ONES:W0_ONES + 32] = 1.0

    # [i, src, tap, o]
    w1 = np.stack([W_cx.transpose(1, 2, 3, 0).reshape(128, 9, 128),
                   W_ch.transpose(1, 2, 3, 0).reshape(128, 9, 128)],
                  axis=1).reshape(128, W1_END)

    w2 = np.zeros((128, W2_END), f)
    # [c, (q|k), b, a]
    w2[:, W2_QK:W2_QK + 1024] = np.stack(
        [W_q.transpose(2, 0, 1), W_k.transpose(2, 0, 1)], axis=1
    ).reshape(128, 1024)
    # [c, srcpair, a-pair]: xc feeds branches (0,2), hc feeds (1,3)
    w2[:, W2_V:W2_V + 512] = np.stack([
        np.concatenate([W_v[0].T, W_v[2].T], axis=1),
        np.concatenate([W_v[1].T, W_v[3].T], axis=1)], axis=1).reshape(128, 512)

    w3 = np.zeros((128, W3_END), f)
    # [a, gate, branch, r]
    w3[:, W3_TOK:W3_TOK + 2048] = W_tok.transpose(3, 0, 1, 2).reshape(128, 2048)
    # [c, gate, src, r]
    w3[:, W3_SKIP:W3_SKIP + 1024] = W_skip.transpose(3, 0, 1, 2).reshape(128, 1024)
    # W_out pre-halved: h = (1+to)*tanh(c)/2 and the /2 lives here
    w3[:, W3_OUT:W3_OUT + 128] = 0.5 * np.asarray(inputs["W_out"], f).T
    # tanh-form biases: gates i,f,o take tanh(y/2 + b/2)
    btok = np.asarray(inputs["b_tok"], f).T  # [R, 4]
    w3[:, W3_BTOK:W3_BTOK + 4] = btok * np.array([0.5, 0.5, 1.0, 0.5], f)
    w3[:, W3_BOUT] = np.asarray(inputs["b_out"], f)

    return {"w0": c(w0), "w1": c(w1), "w2": c(w2), "w3": c(w3)}


def kernel(**inputs):
    from concourse.bass_utils import run_bass_kernel_spmd
    if "nc" not in _CACHE:
        _CACHE["nc"] = _build_program()
    nc = _CACHE["nc"]
    f = np.float32
    x = np.asarray(inputs["x"], f).reshape(N, I, HW)
    hp = np.asarray(inputs["h_prev"], f).reshape(N, R, HW)
    cp = np.asarray(inputs["c_prev"], f).reshape(N, R, HW)
    shared = _prep_shared(inputs)
    in_maps = []
    for ci in range(NCORES):
        sl = slice(S * ci, S * ci + S)
        m = dict(shared)
        m["xin"] = np.ascontiguousarray(x[sl])
        m["hin"] = np.ascontiguousarray(hp[sl])
        m["cin"] = np.ascontiguousarray(cp[sl])
        in_maps.append(m)
    res = run_bass_kernel_spmd(nc, in_maps, core_ids=list(range(NCORES)))
    y = np.concatenate([r["yout"].reshape(S, R, H, W) for r in res.results],
                       axis=0)
    return y.astype(np.float32)


# revision 44
# speedup vs baseline: 1.2731x; 1.2731x over previous
# Trainium2 Bass kernel for nn_CvtLstm: ConvLSTM cell with 4-branch,
# 4-head spatial attention. Data-parallel over batch N=32 across 8
# NeuronCores (4 samples per core); weights replicated to every core.
#
# Per-core layout: channels on partitions, flattened 16x16 spatial (256)
# on the free dim. conv3x3 = 9 shifted matmuls reading a zero-padded
# [128, 2, 18, 18] tile. Attention scores are computed directly in the
# transposed [d, q] layout (lhsT = per-head k rows, K=32 row-partial
# matmuls); exp on the ACT engine with no max subtraction (scores lie in
# [-9, 8]); the PV product and the softmax denominator Z come from one
# M=64 matmul per (head, d-chunk) whose weight columns are [vT_g | ones].
#
# Engine budget decisions (from TimelineSim trace analysis):
# - No per-iter DMAs: the old head-restack DMAs serialized on the single
#   HWDGE queue (625ns/descriptor). Normalization is now: two PSUM->SBUF
#   extraction copies (a-rows on DVE, z-rows alternating DVE/ACT) plus 4
#   partition-scattering divides on the otherwise-idle Pool engine
#   (GPSIMD cannot read PSUM; both-SBUF operands must share a base
#   partition, hence both extracts land at base 0).
# - All gates use Tanh (sigmoid(x) = (1+tanh(x/2))/2 folded into scales/
#   weights) so the single act table set {exp, tanh, copy} is loaded
#   once; sigmoid and exp never coexist in one table set.
# - LSTM update is 4 fused scalar_tensor_tensor ops: (t+1)*x computes
#   2*gate*x without materializing the 0.5s, which are folded into
#   tanh-scale and W_out.
# - Zero-padding and the vT ones-columns are Pool memsets, not DMAs.
# - Weights ship in 4 packed DMAs ordered by first use (winT -> conv ->
#   q/k/v -> gates/out) so the PE pipeline starts ~1.5us in.
#
# Hardware constraint honored throughout: two row-partial matmuls at
# different row groups back-to-back fault the device; full-row dummy
# separators are inserted at group changes (see baseline notes).

import numpy as np

N, I, H, W = 32, 64, 16, 16
R, CM, A, HEADS, HC = 128, 128, 128, 4, 32
HW = H * W           # 256
S = 4                # samples per core
NCORES = 8

_CACHE = {}

# packed weight column offsets (f32 words per partition)
W0_WIN, W0_BIN, W0_ONES, W0_END = 0, 128, 129, 161
W1_CONV, W1_END = 0, 2304
W2_QK, W2_V, W2_END = 0, 1024, 1536
W3_TOK, W3_SKIP, W3_OUT, W3_BTOK, W3_BOUT, W3_END = 0, 2048, 3072, 3200, 3204, 3205


def _build_program():
    import contextlib
    import concourse.bacc as bacc
    import concourse.mybir as mybir
    import concourse.tile as tile

    import concourse.bass_isa as bass_isa

    F32 = mybir.dt.float32
    F32R = mybir.dt.float32r
    BF16 = mybir.dt.bfloat16
    AF = mybir.ActivationFunctionType
    ALU = mybir.AluOpType
    RADD = bass_isa.ReduceOp.add

    nc = bacc.Bacc("TRN2", target_bir_lowering=False, debug=False)

    def dram(name, shape, kind="ExternalInput"):
        return nc.dram_tensor(name, list(shape), F32, kind=kind).ap()

    xin = dram("xin", [S, I, HW])
    hin = dram("hin", [S, R, HW])
    cin = dram("cin", [S, R, HW])
    w0d = dram("w0", [128, W0_END])
    w1d = dram("w1", [128, W1_END])
    w2d = dram("w2", [128, W2_END])
    w3d = dram("w3", [128, W3_END])
    yout = dram("yout", [S, R, HW], kind="ExternalOutput")

    QSRC = [0, 0, 1, 1]   # q source per branch: 0=xc, 1=hc
    KSRC = [0, 1, 0, 1]   # k/v source per branch
    BORDER = [3, 1, 2, 0]  # per-pass branch order (b3 = pure hc, earliest)
    GSCALE = [0.5, 0.5, 1.0, 0.5]  # tanh input scale per gate (i,f,g,o)

    with tile.TileContext(nc) as tc:
        with contextlib.ExitStack() as ctx:
            wpool = ctx.enter_context(tc.tile_pool(name="wts", bufs=1))
            sbA = ctx.enter_context(tc.tile_pool(name="sbA", bufs=2))
            sbB = ctx.enter_context(tc.tile_pool(name="sbB", bufs=2))
            epool = ctx.enter_context(tc.tile_pool(name="ep", bufs=3))
            stp = ctx.enter_context(tc.tile_pool(name="st", bufs=2, space="PSUM"))
            azp = ctx.enter_context(tc.tile_pool(name="az", bufs=2, space="PSUM"))
            pwp = ctx.enter_context(tc.tile_pool(name="pw", bufs=2, space="PSUM"))

            # ---------------- input DMAs for pass 0 (issued first so the
            # XT matmul can start while the big weight DMAs stream) -----
            h_pad0 = sbA.tile([128, 648], F32R, tag="hpad", name="hpad")
            xt_pad0 = sbA.tile([128, 648], F32R, tag="xtpad", name="xtpad")
            x2_0 = sbA.tile([64, 2, 256], F32R, tag="x2", name="x2")
            nc.gpsimd.memset(h_pad0.bitcast(F32), 0.0)
            nc.gpsimd.memset(xt_pad0.bitcast(F32), 0.0)
            hv0 = h_pad0.rearrange("p (s y x) -> p s y x", s=2, y=18, x=18)
            for s in range(2):
                nc.sync.dma_start(
                    out=hv0[:, s, 1:17, 1:17],
                    in_=hin[s].rearrange("c (h w) -> c h w", h=16).bitcast(F32R))
            nc.sync.dma_start(
                out=x2_0, in_=xin[0:2].rearrange("s c q -> c s q").bitcast(F32R))

            # ---------------- weights to SBUF (4 packed DMAs) ----------
            def wload(name, src, w):
                t = wpool.tile([128, w], F32R, tag=name, name=name)
                nc.sync.dma_start(out=t, in_=src.bitcast(F32R))
                return t

            w0_s = wload("w0", w0d, W0_END)
            w1_s = wload("w1", w1d, W1_END)
            w2_s = wload("w2", w2d, W2_END)
            w3_s = wload("w3", w3d, W3_END)

            winT_s = w0_s[0:64, W0_WIN:W0_WIN + 128]
            b_in_s = w0_s[:, W0_BIN:W0_BIN + 1].bitcast(F32)
            ones_s = w0_s[:, W0_ONES:W0_ONES + 32]
            wconvT_s = w1_s[:, W1_CONV:W1_CONV + 2304].rearrange(
                "p (s t o) -> p s t o", s=2, t=9)
            wqkT_s = w2_s[:, W2_QK:W2_QK + 1024].rearrange(
                "p (k b o) -> p k b o", k=2, b=4)
            wvT_s = w2_s[:, W2_V:W2_V + 512].rearrange("p (s q) -> p s q", s=2)
            wtokT_s = w3_s[:, W3_TOK:W3_TOK + 2048].rearrange(
                "p (g b o) -> p g b o", g=4, b=4)
            wskipT_s = w3_s[:, W3_SKIP:W3_SKIP + 1024].rearrange(
                "p (g s o) -> p g s o", g=4, s=2)
            woutT_s = w3_s[:, W3_OUT:W3_OUT + 128]
            btok_s = w3_s[:, W3_BTOK:W3_BTOK + 4].bitcast(F32)
            bout_s = w3_s[:, W3_BOUT:W3_BOUT + 1].bitcast(F32)

            # ---------------- per-pass state ----------------
            xc_sb = [None, None]
            hc_sb = [None, None]
            q_sb = [[None] * 4, [None] * 4]
            k_sb = [[None] * 4, [None] * 4]
            vt_sb = [[None] * 4, [None] * 4]
            a_all = [None, None]
            cprev_sb = [None, None]
            gate_sb = [[None] * 4, [None] * 4]

            def emit_input_pads(p):
                """zero-padded xt/h tiles + x2 + XT matmul + tanh.
                For p=0 the pads/DMAs were already emitted above."""
                if p == 0:
                    xt_pad, h_pad, x2 = xt_pad0, h_pad0, x2_0
                else:
                    h_pad = sbA.tile([128, 648], F32R, tag="hpad", name="hpad")
                    xt_pad = sbA.tile([128, 648], F32R, tag="xtpad", name="xtpad")
                    x2 = sbA.tile([64, 2, 256], F32R, tag="x2", name="x2")
                    nc.gpsimd.memset(h_pad.bitcast(F32), 0.0)
                    nc.gpsimd.memset(xt_pad.bitcast(F32), 0.0)
                    hv = h_pad.rearrange("p (s y x) -> p s y x", s=2, y=18, x=18)
                    for s in range(2):
                        nc.sync.dma_start(
                            out=hv[:, s, 1:17, 1:17],
                            in_=hin[2 * p + s].rearrange(
                                "c (h w) -> c h w", h=16).bitcast(F32R))
                    nc.sync.dma_start(
                        out=x2,
                        in_=xin[2 * p:2 * p + 2].rearrange(
                            "s c q -> c s q").bitcast(F32R))
                XT = pwp.tile([128, 512], F32, tag="pw", name="XT")
                nc.tensor.matmul(out=XT, lhsT=winT_s,
                                 rhs=x2.rearrange("p s q -> p (s q)"),
                                 start=True, stop=True)
                xv = xt_pad.rearrange("p (s y x) -> p s y x", s=2, y=18, x=18)
                nc.scalar.activation(
                    out=xv[:, :, 1:17, 1:17],
                    in_=XT.rearrange("p (s h w) -> p s h w", s=2, h=16, w=16),
                    func=AF.Tanh, bias=b_in_s)
                return xt_pad, h_pad

            def emit_conv(p, src, pad):
                """3x3 SAME conv via 9 shifted matmuls; src 0=xc, 1=hc."""
                CP = pwp.tile([128, 512], F32, tag="pw", name="CP")
                pv = pad.rearrange("p (s y x) -> p s y x", s=2, y=18, x=18)
                for t in range(9):
                    ky, kx = divmod(t, 3)
                    nc.tensor.matmul(out=CP, lhsT=wconvT_s[:, src, t, :],
                                     rhs=pv[:, :, ky:ky + 16, kx:kx + 16],
                                     start=(t == 0), stop=(t == 8))
                dst = sbA.tile([128, 512], F32R, tag=("xc" if src == 0 else "hc"),
                               name=("xc" if src == 0 else "hc"))
                nc.vector.tensor_copy(dst, CP)
                if src == 0:
                    xc_sb[p] = dst
                else:
                    hc_sb[p] = dst

            def emit_qk(p, b):
                srcq = xc_sb[p] if QSRC[b] == 0 else hc_sb[p]
                srck = xc_sb[p] if KSRC[b] == 0 else hc_sb[p]
                QB = pwp.tile([128, 512], F32, tag="pw", name="QB")
                nc.tensor.matmul(out=QB, lhsT=wqkT_s[:, 0, b, :], rhs=srcq,
                                 start=True, stop=True)
                q_sb[p][b] = sbB.tile([128, 512], BF16, tag=f"q{b}", name=f"q{b}")
                nc.vector.tensor_copy(q_sb[p][b], QB)
                KB = pwp.tile([128, 512], F32, tag="pw", name="KB")
                nc.tensor.matmul(out=KB, lhsT=wqkT_s[:, 1, b, :], rhs=srck,
                                 start=True, stop=True)
                k_sb[p][b] = sbB.tile([128, 512], BF16, tag=f"k{b}", name=f"k{b}")
                nc.vector.tensor_copy(k_sb[p][b], KB)

            def emit_vt(p, src):
                """vT for the two branches fed by `src`:
                vt_sb[b] = [128, (sc 4, g 4, ch 32)] bf16, v-only (Z comes
                from the pT-side partition reduce, not from ones columns)."""
                b0 = src            # branches (0,2) from xc, (1,3) from hc
                for b in (b0, b0 + 2):
                    if vt_sb[p][b] is None:
                        vt_sb[p][b] = sbB.tile([128, 512], BF16,
                                               tag=f"vt{b}", name=f"vt{b}")
                src_sb = xc_sb[p] if src == 0 else hc_sb[p]
                sv = src_sb.rearrange("p (s c d) -> p s c d", s=2, c=2)
                for s in range(2):
                    for c in range(2):
                        VT = pwp.tile([128, 256], F32, tag="pw", name="VT")
                        nc.tensor.matmul(out=VT, lhsT=sv[:, s, c, :],
                                         rhs=wvT_s[:, src, :],
                                         start=True, stop=True)
                        sc = s * 2 + c
                        for j, b in enumerate((b0, b0 + 2)):
                            nc.vector.tensor_copy(
                                vt_sb[p][b][:, sc * 128:(sc + 1) * 128],
                                VT[:, j * 128:(j + 1) * 128])

            def emit_cprev(p):
                cprev_sb[p] = sbA.tile([128, 512], F32, tag="cprev", name="cprev")
                nc.sync.dma_start(
                    out=cprev_sb[p],
                    in_=cin[2 * p:2 * p + 2].rearrange("s c q -> c s q"))

            # ---------------- attention iteration pieces ----------------
            def emit_scores_exp(p, b, s):
                """returns pT tile [128, 2048] bf16 = exp(scores^T), layout
                (g, c, q) 4x2x256. One ST bank + one exp per head so PV(g)
                can start as soon as exp(g) lands."""
                kv = k_sb[p][b].rearrange("p (s c d) -> p s c d", s=2, c=2)
                qv = q_sb[p][b].rearrange("p (s q) -> p s q", s=2)
                pT = sbB.tile([128, 2048], BF16, tag="pt", name="pT")
                for h in range(2):
                    ST = stp.tile([128, 1024], F32, tag="st", name="ST")
                    for gg in range(2):
                        g = 2 * h + gg
                        if (h, gg) != (0, 0):
                            dsl = ST[0:32, 512:544] if gg == 1 else ST[0:32, 0:32]
                            nc.tensor.matmul(out=dsl, lhsT=ones_s,
                                             rhs=ones_s, start=True, stop=True,
                                             skip_group_check=True)
                        for c in range(2):
                            nc.tensor.matmul(
                                out=ST[:, gg * 512 + c * 256:gg * 512 + c * 256 + 256],
                                lhsT=kv[32 * g:32 * g + 32, s, c, :],
                                rhs=qv[32 * g:32 * g + 32, s, :],
                                start=True, stop=True, skip_group_check=True,
                                tile_position=(32 * g, 0))
                    nc.scalar.activation(out=pT[:, h * 1024:(h + 1) * 1024],
                                         in_=ST, func=AF.Exp)
                return pT

            def emit_zpath(pT):
                """1/Z from pT alone (runs concurrently with the PV matmuls):
                c-fold on DVE (bf16 fast mode), per-head partition reduce on
                Pool, row assembly on Pool, one DVE reciprocal. Returns rz
                [128, 256] f32 with rows 32g:32g+32 = 1/z_g."""
                pc = epool.tile([128, 1024], BF16, tag="pc", name="pc")
                pv4 = pT.rearrange("p (g c q) -> p g c q", g=4, c=2)
                for g in range(4):
                    nc.vector.tensor_add(pc[:, g * 256:(g + 1) * 256],
                                         pv4[:, g, 0, :], pv4[:, g, 1, :])
                zr = epool.tile([128, 1024], F32, tag="zr", name="zr")
                for g in range(4):
                    nc.gpsimd.partition_all_reduce(
                        zr[:, g * 256:(g + 1) * 256],
                        pc[:, g * 256:(g + 1) * 256],
                        channels=128, reduce_op=RADD)
                zasm = epool.tile([128, 256], F32, tag="zasm", name="zasm")
                for g in range(4):
                    src = zr[32 * g:32 * g + 32, g * 256:(g + 1) * 256]
                    dst = zasm[32 * g:32 * g + 32, :]
                    if g < 2:
                        nc.gpsimd.tensor_copy(dst, src)
                    elif g == 2:
                        nc.scalar.activation(out=dst, in_=src, func=AF.Copy)
                    else:
                        nc.vector.tensor_copy(dst, src)
                rz = epool.tile([128, 256], F32, tag="rz", name="rz")
                nc.vector.reciprocal_approx_fast(out=rz, in_=zasm)
                return rz

            def emit_pvz(p, b, s, pT):
                # A lands PSUM-aligned: head g at partitions 32g:32g+32 of a
                # single [128, 256] tile (half a PSUM bank), M=32 per matmul.
                AZ = azp.tile([128, 256], F32, tag="az", name="AZ")
                for g in range(4):
                    for c in range(2):
                        sc = s * 2 + c
                        nc.tensor.matmul(
                            out=AZ[32 * g:32 * g + 32, :],
                            lhsT=vt_sb[p][b][:, sc * 128 + 32 * g:sc * 128 + 32 * g + 32],
                            rhs=pT[:, g * 512 + c * 256:g * 512 + c * 256 + 256],
                            start=(c == 0), stop=(c == 1), skip_group_check=True,
                            tile_position=(0, 32 * g))
                return AZ

            def emit_norm(p, b, s, AZ, rz):
                """a_all[:, slot] = AZ * rz — a single DVE multiply (AZ is
                already partition-aligned, rz precomputed by the z-path)."""
                slot = b * 2 + s
                if a_all[p] is None:
                    a_all[p] = sbA.tile([128, 2048], F32R, tag="aall", name="a_all")
                nc.vector.tensor_mul(
                    a_all[p][:, slot * 256:(slot + 1) * 256], AZ, rz)

            # ---------------- gates / state / output ----------------
            def emit_gate(p, gi):
                G = pwp.tile([128, 512], F32, tag="pw", name="G")
                av = a_all[p].rearrange("p (b s q) -> p b (s q)", b=4, s=2)
                for b in range(4):
                    nc.tensor.matmul(out=G, lhsT=wtokT_s[:, gi, b, :],
                                     rhs=av[:, b, :],
                                     start=(b == 0), stop=False)
                nc.tensor.matmul(out=G, lhsT=wskipT_s[:, gi, 0, :],
                                 rhs=xc_sb[p], start=False, stop=False)
                nc.tensor.matmul(out=G, lhsT=wskipT_s[:, gi, 1, :],
                                 rhs=hc_sb[p], start=False, stop=True)
                gate_sb[p][gi] = sbA.tile([128, 512], F32, tag=f"gate{gi}",
                                          name=f"gate{gi}")
                # all gates via Tanh: sigmoid(y) = (1+tanh(y/2))/2; the /2 is
                # in GSCALE + host-halved biases, the (1+t)/2 in the update.
                nc.scalar.activation(out=gate_sb[p][gi], in_=G, func=AF.Tanh,
                                     scale=GSCALE[gi], bias=btok_s[:, gi:gi + 1])

            def emit_update_out(p, tail=False):
                ti, tf, tg, to = gate_sb[p]
                # c2 = 2c = (1+tf)*c_prev + (1+ti)*g  (stt is DVE-only).
                # In the final-pass tail every op is latency-critical, so the
                # plain add runs on DVE too; mid-schedule it goes to Pool.
                s1 = sbA.tile([128, 512], F32, tag="fc", name="s1")
                nc.vector.scalar_tensor_tensor(
                    out=s1, in0=tf, scalar=1.0, in1=cprev_sb[p],
                    op0=ALU.add, op1=ALU.mult)
                s2 = sbA.tile([128, 512], F32, tag="ig", name="s2")
                nc.vector.scalar_tensor_tensor(
                    out=s2, in0=ti, scalar=1.0, in1=tg,
                    op0=ALU.add, op1=ALU.mult)
                c2 = sbA.tile([128, 512], F32, tag="c", name="c2")
                (nc.vector if tail else nc.gpsimd).tensor_add(c2, s1, s2)
                # tanh(c) = tanh(c2/2); h2 = 2h = (1+to)*tanh(c); W_out is
                # pre-halved on the host to absorb the remaining 1/2.
                tcs = sbA.tile([128, 512], F32, tag="tc", name="tcs")
                nc.scalar.activation(out=tcs, in_=c2, func=AF.Tanh, scale=0.5)
                hs = sbA.tile([128, 512], F32R, tag="h", name="hs")
                nc.vector.scalar_tensor_tensor(
                    out=hs, in0=to, scalar=1.0, in1=tcs,
                    op0=ALU.add, op1=ALU.mult)
                OUT = pwp.tile([128, 512], F32, tag="pw", name="OUT")
                nc.tensor.matmul(out=OUT, lhsT=woutT_s, rhs=hs,
                                 start=True, stop=True)
                osb = sbA.tile([128, 512], F32, tag="out", name="osb")
                nc.scalar.activation(out=osb, in_=OUT, func=AF.Identity,
                                     bias=bout_s[:, 0:1])
                nc.sync.dma_start(
                    out=yout[2 * p:2 * p + 2].rearrange("s c q -> c s q"),
                    in_=osb.rearrange("p (s q) -> p s q", s=2))

            # ---------------- emission schedule ----------------
            # prologue: pass-0 essentials up to branch 3 (pure hc)
            xt_pad0r, h_pad0r = emit_input_pads(0)
            emit_conv(0, 1, h_pad0r)     # hc pass0
            emit_qk(0, 3)
            emit_vt(0, 1)                # vT for b1, b3 (hc source)
            pads1 = [None]

            def filler(i):
                if i == 0:
                    emit_conv(0, 0, xt_pad0r)         # xc pass0
                elif i == 1:
                    emit_qk(0, 1)
                    emit_qk(0, 2)
                elif i == 2:
                    emit_qk(0, 0)
                    emit_vt(0, 0)
                    emit_cprev(0)
                elif i == 3:
                    pads1[0] = emit_input_pads(1)
                elif i == 4:
                    emit_conv(1, 1, pads1[0][1])      # hc pass1
                elif i == 5:
                    emit_conv(1, 0, pads1[0][0])      # xc pass1
                elif i == 6:
                    emit_qk(1, 3)
                    emit_vt(1, 1)
                elif i == 7:
                    emit_qk(1, 1)
                    emit_qk(1, 2)
                elif i == 8:
                    emit_qk(1, 0)
                    emit_vt(1, 0)
                    emit_cprev(1)
                elif i in (9, 10, 11, 12):
                    emit_gate(0, i - 9)
                elif i == 13:
                    emit_update_out(0)

            iters = [(p, b, s) for p in (0, 1) for b in BORDER for s in (0, 1)]
            prev = None
            for i, (p, b, s) in enumerate(iters):
                pT = emit_scores_exp(p, b, s)
                rz = emit_zpath(pT)
                if prev is not None:
                    pp, pb, ps, ppT, prz = prev
                    AZ = emit_pvz(pp, pb, ps, ppT)
                    emit_norm(pp, pb, ps, AZ, prz)
                prev = (p, b, s, pT, rz)
                filler(i)
            pp, pb, ps, ppT, prz = prev
            AZ = emit_pvz(pp, pb, ps, ppT)
            emit_norm(pp, pb, ps, AZ, prz)
            # tail: order gates so the update's dependency chain starts early
            # (f and g feed c2; o is only needed after tanh(c))
            for gi in (1, 2, 0, 3):
                emit_gate(1, gi)
            emit_update_out(1, tail=True)

    nc.compile()
    return nc


def _prep_shared(inputs):
    f = np.float32
    c = np.ascontiguousarray
    W_cx, W_ch = np.asarray(inputs["W_cx"], f), np.asarray(inputs["W_ch"], f)
    W_q, W_k, W_v = (np.asarray(inputs[k], f) for k in ("W_q", "W_k", "W_v"))
    W_tok, W_skip = np.asarray(inputs["W_tok"], f), np.asarray(inputs["W_skip"], f)

    w0 = np.zeros((128, W0_END), f)
    w0[0:64, W0_WIN:W0_WIN + 128] = np.asarray(inputs["W_in"], f).T
    w0[:, W0_BIN] = np.asarray(inputs["b_in"], f)
    w0[:, W0_ONES:W0_ONES + 32] = 1.0

    # [i, src, tap, o]
    w1 = np.stack([W_cx.transpose(1, 2, 3, 0).reshape(128, 9, 128),
                   W_ch.transpose(1, 2, 3, 0).reshape(128, 9, 128)],
                  axis=1).reshape(128, W1_END)

    w2 = np.zeros((128, W2_END), f)
    # [c, (q|k), b, a]
    w2[:, W2_QK:W2_QK + 1024] = np.stack(
        [W_q.transpose(2, 0, 1), W_k.transpose(2, 0, 1)], axis=1
    ).reshape(128, 1024)
    # [c, srcpair, a-pair]: xc feeds branches (0,2), hc feeds (1,3)
    w2[:, W2_V:W2_V + 512] = np.stack([
        np.concatenate([W_v[0].T, W_v[2].T], axis=1),
        np.concatenate([W_v[1].T, W_v[3].T], axis=1)], axis=1).reshape(128, 512)

    w3 = np.zeros((128, W3_END), f)
    # [a, gate, branch, r]
    w3[:, W3_TOK:W3_TOK + 2048] = W_tok.transpose(3, 0, 1, 2).reshape(128, 2048)
    # [c, gate, src, r]
    w3[:, W3_SKIP:W3_SKIP + 1024] = W_skip.transpose(3, 0, 1, 2).reshape(128, 1024)
    # W_out pre-halved: h = (1+to)*tanh(c)/2 and the /2 lives here
    w3[:, W3_OUT:W3_OUT + 128] = 0.5 * np.asarray(inputs["W_out"], f).T
    # tanh-form biases: gates i,f,o take tanh(y/2 + b/2)
    btok = np.asarray(inputs["b_tok"], f).T  # [R, 4]
    w3[:, W3_BTOK:W3_BTOK + 4] = btok * np.array([0.5, 0.5, 1.0, 0.5], f)
    w3[:, W3_BOUT] = np.asarray(inputs["b_out"], f)

    return {"w0": c(w0), "w1": c(w1), "w2": c(w2), "w3": c(w3)}


def kernel(**inputs):
    from concourse.bass_utils import run_bass_kernel_spmd
    if "nc" not in _CACHE:
        _CACHE["nc"] = _build_program()
    nc = _CACHE["nc"]
    f = np.float32
    x = np.asarray(inputs["x"], f).reshape(N, I, HW)
    hp = np.asarray(inputs["h_prev"], f).reshape(N, R, HW)
    cp = np.asarray(inputs["c_prev"], f).reshape(N, R, HW)
    shared = _prep_shared(inputs)
    in_maps = []
    for ci in range(NCORES):
        sl = slice(S * ci, S * ci + S)
        m = dict(shared)
        m["xin"] = np.ascontiguousarray(x[sl])
        m["hin"] = np.ascontiguousarray(hp[sl])
        m["cin"] = np.ascontiguousarray(cp[sl])
        in_maps.append(m)
    res = run_bass_kernel_spmd(nc, in_maps, core_ids=list(range(NCORES)))
    y = np.concatenate([r["yout"].reshape(S, R, H, W) for r in res.results],
                       axis=0)
    return y.astype(np.float32)


# revision 47
# speedup vs baseline: 1.2799x; 1.0053x over previous
# Trainium2 Bass kernel for nn_CvtLstm: ConvLSTM cell with 4-branch,
# 4-head spatial attention. Data-parallel over batch N=32 across 8
# NeuronCores (4 samples per core); weights replicated to every core.
#
# Per-core layout: channels on partitions, flattened 16x16 spatial (256)
# on the free dim. conv3x3 = 9 shifted matmuls reading a zero-padded
# [128, 2, 18, 18] tile. Attention scores are computed directly in the
# transposed [d, q] layout (lhsT = per-head k rows, K=32 row-partial
# matmuls); exp on the ACT engine with no max subtraction (scores lie in
# [-9, 8]); the PV product and the softmax denominator Z come from one
# M=64 matmul per (head, d-chunk) whose weight columns are [vT_g | ones].
#
# Engine budget decisions (from TimelineSim trace analysis):
# - No per-iter DMAs: the old head-restack DMAs serialized on the single
#   HWDGE queue (625ns/descriptor). Normalization is now: two PSUM->SBUF
#   extraction copies (a-rows on DVE, z-rows alternating DVE/ACT) plus 4
#   partition-scattering divides on the otherwise-idle Pool engine
#   (GPSIMD cannot read PSUM; both-SBUF operands must share a base
#   partition, hence both extracts land at base 0).
# - All gates use Tanh (sigmoid(x) = (1+tanh(x/2))/2 folded into scales/
#   weights) so the single act table set {exp, tanh, copy} is loaded
#   once; sigmoid and exp never coexist in one table set.
# - LSTM update is 4 fused scalar_tensor_tensor ops: (t+1)*x computes
#   2*gate*x without materializing the 0.5s, which are folded into
#   tanh-scale and W_out.
# - Zero-padding and the vT ones-columns are Pool memsets, not DMAs.
# - Weights ship in 4 packed DMAs ordered by first use (winT -> conv ->
#   q/k/v -> gates/out) so the PE pipeline starts ~1.5us in.
#
# Hardware constraint honored throughout: two row-partial matmuls at
# different row groups back-to-back fault the device; full-row dummy
# separators are inserted at group changes (see baseline notes).

import numpy as np

N, I, H, W = 32, 64, 16, 16
R, CM, A, HEADS, HC = 128, 128, 128, 4, 32
HW = H * W           # 256
S = 4                # samples per core
NCORES = 8

_CACHE = {}

# packed weight column offsets (f32 words per partition)
W0_WIN, W0_BIN, W0_ONES, W0_END = 0, 128, 129, 161
W1_CONV, W1_END = 0, 2304
W2_QK, W2_V, W2_END = 0, 1024, 1536
W3_TOK, W3_SKIP, W3_OUT, W3_BTOK, W3_BOUT, W3_END = 0, 2048, 3072, 3200, 3204, 3205


def _build_program():
    import contextlib
    import concourse.bacc as bacc
    import concourse.mybir as mybir
    import concourse.tile as tile

    import concourse.bass_isa as bass_isa

    F32 = mybir.dt.float32
    F32R = mybir.dt.float32r
    BF16 = mybir.dt.bfloat16
    AF = mybir.ActivationFunctionType
    ALU = mybir.AluOpType
    RADD = bass_isa.ReduceOp.add

    nc = bacc.Bacc("TRN2", target_bir_lowering=False, debug=False)

    def dram(name, shape, kind="ExternalInput"):
        return nc.dram_tensor(name, list(shape), F32, kind=kind).ap()

    xin = dram("xin", [S, I, HW])
    hin = dram("hin", [S, R, HW])
    cin = dram("cin", [S, R, HW])
    w0d = dram("w0", [128, W0_END])
    w1d = dram("w1", [128, W1_END])
    w2d = dram("w2", [128, W2_END])
    w3d = dram("w3", [128, W3_END])
    yout = dram("yout", [S, R, HW], kind="ExternalOutput")

    QSRC = [0, 0, 1, 1]   # q source per branch: 0=xc, 1=hc
    KSRC = [0, 1, 0, 1]   # k/v source per branch
    BORDER = [3, 1, 2, 0]  # per-pass branch order (b3 = pure hc, earliest)
    GSCALE = [0.5, 0.5, 1.0, 0.5]  # tanh input scale per gate (i,f,g,o)

    with tile.TileContext(nc) as tc:
        with contextlib.ExitStack() as ctx:
            wpool = ctx.enter_context(tc.tile_pool(name="wts", bufs=1))
            sbA = ctx.enter_context(tc.tile_pool(name="sbA", bufs=2))
            sbB = ctx.enter_context(tc.tile_pool(name="sbB", bufs=2))
            epool = ctx.enter_context(tc.tile_pool(name="ep", bufs=3))
            stp = ctx.enter_context(tc.tile_pool(name="st", bufs=2, space="PSUM"))
            azp = ctx.enter_context(tc.tile_pool(name="az", bufs=2, space="PSUM"))
            pwp = ctx.enter_context(tc.tile_pool(name="pw", bufs=2, space="PSUM"))

            # ---------------- input DMAs for pass 0 (issued first so the
            # XT matmul can start while the big weight DMAs stream) -----
            h_pad0 = sbA.tile([128, 648], F32R, tag="hpad", name="hpad")
            xt_pad0 = sbA.tile([128, 648], F32R, tag="xtpad", name="xtpad")
            x2_0 = sbA.tile([64, 2, 256], F32R, tag="x2", name="x2")
            nc.gpsimd.memset(h_pad0.bitcast(F32), 0.0)
            nc.gpsimd.memset(xt_pad0.bitcast(F32), 0.0)
            hv0 = h_pad0.rearrange("p (s y x) -> p s y x", s=2, y=18, x=18)
            for s in range(2):
                nc.sync.dma_start(
                    out=hv0[:, s, 1:17, 1:17],
                    in_=hin[s].rearrange("c (h w) -> c h w", h=16).bitcast(F32R))
            nc.sync.dma_start(
                out=x2_0, in_=xin[0:2].rearrange("s c q -> c s q").bitcast(F32R))

            # ---------------- weights to SBUF (4 packed DMAs) ----------
            def wload(name, src, w):
                t = wpool.tile([128, w], F32R, tag=name, name=name)
                nc.sync.dma_start(out=t, in_=src.bitcast(F32R))
                return t

            w0_s = wload("w0", w0d, W0_END)
            w1_s = wload("w1", w1d, W1_END)
            w2_s = wload("w2", w2d, W2_END)
            w3_s = wload("w3", w3d, W3_END)

            winT_s = w0_s[0:64, W0_WIN:W0_WIN + 128]
            b_in_s = w0_s[:, W0_BIN:W0_BIN + 1].bitcast(F32)
            ones_s = w0_s[:, W0_ONES:W0_ONES + 32]
            wconvT_s = w1_s[:, W1_CONV:W1_CONV + 2304].rearrange(
                "p (s t o) -> p s t o", s=2, t=9)
            wqkT_s = w2_s[:, W2_QK:W2_QK + 1024].rearrange(
                "p (k b o) -> p k b o", k=2, b=4)
            wvT_s = w2_s[:, W2_V:W2_V + 512].rearrange("p (s q) -> p s q", s=2)
            wtokT_s = w3_s[:, W3_TOK:W3_TOK + 2048].rearrange(
                "p (g b o) -> p g b o", g=4, b=4)
            wskipT_s = w3_s[:, W3_SKIP:W3_SKIP + 1024].rearrange(
                "p (g s o) -> p g s o", g=4, s=2)
            woutT_s = w3_s[:, W3_OUT:W3_OUT + 128]
            btok_s = w3_s[:, W3_BTOK:W3_BTOK + 4].bitcast(F32)
            bout_s = w3_s[:, W3_BOUT:W3_BOUT + 1].bitcast(F32)

            # ---------------- per-pass state ----------------
            xc_sb = [None, None]
            hc_sb = [None, None]
            q_sb = [[None] * 4, [None] * 4]
            k_sb = [[None] * 4, [None] * 4]
            vt_sb = [[None] * 4, [None] * 4]
            a_all = [None, None]
            cprev_sb = [None, None]
            gate_sb = [[None] * 4, [None] * 4]

            def emit_input_pads(p):
                """zero-padded xt/h tiles + x2 + XT matmul + tanh.
                For p=0 the pads/DMAs were already emitted above."""
                if p == 0:
                    xt_pad, h_pad, x2 = xt_pad0, h_pad0, x2_0
                else:
                    h_pad = sbA.tile([128, 648], F32R, tag="hpad", name="hpad")
                    xt_pad = sbA.tile([128, 648], F32R, tag="xtpad", name="xtpad")
                    x2 = sbA.tile([64, 2, 256], F32R, tag="x2", name="x2")
                    nc.gpsimd.memset(h_pad.bitcast(F32), 0.0)
                    nc.gpsimd.memset(xt_pad.bitcast(F32), 0.0)
                    hv = h_pad.rearrange("p (s y x) -> p s y x", s=2, y=18, x=18)
                    for s in range(2):
                        nc.sync.dma_start(
                            out=hv[:, s, 1:17, 1:17],
                            in_=hin[2 * p + s].rearrange(
                                "c (h w) -> c h w", h=16).bitcast(F32R))
                    nc.sync.dma_start(
                        out=x2,
                        in_=xin[2 * p:2 * p + 2].rearrange(
                            "s c q -> c s q").bitcast(F32R))
                XT = pwp.tile([128, 512], F32, tag="pw", name="XT")
                nc.tensor.matmul(out=XT, lhsT=winT_s,
                                 rhs=x2.rearrange("p s q -> p (s q)"),
                                 start=True, stop=True)
                xv = xt_pad.rearrange("p (s y x) -> p s y x", s=2, y=18, x=18)
                nc.scalar.activation(
                    out=xv[:, :, 1:17, 1:17],
                    in_=XT.rearrange("p (s h w) -> p s h w", s=2, h=16, w=16),
                    func=AF.Tanh, bias=b_in_s)
                return xt_pad, h_pad

            def emit_conv(p, src, pad):
                """3x3 SAME conv via 9 shifted matmuls; src 0=xc, 1=hc."""
                CP = pwp.tile([128, 512], F32, tag="pw", name="CP")
                pv = pad.rearrange("p (s y x) -> p s y x", s=2, y=18, x=18)
                for t in range(9):
                    ky, kx = divmod(t, 3)
                    nc.tensor.matmul(out=CP, lhsT=wconvT_s[:, src, t, :],
                                     rhs=pv[:, :, ky:ky + 16, kx:kx + 16],
                                     start=(t == 0), stop=(t == 8))
                dst = sbA.tile([128, 512], F32R, tag=("xc" if src == 0 else "hc"),
                               name=("xc" if src == 0 else "hc"))
                nc.vector.tensor_copy(dst, CP)
                if src == 0:
                    xc_sb[p] = dst
                else:
                    hc_sb[p] = dst

            def emit_qk(p, b):
                srcq = xc_sb[p] if QSRC[b] == 0 else hc_sb[p]
                srck = xc_sb[p] if KSRC[b] == 0 else hc_sb[p]
                QB = pwp.tile([128, 512], F32, tag="pw", name="QB")
                nc.tensor.matmul(out=QB, lhsT=wqkT_s[:, 0, b, :], rhs=srcq,
                                 start=True, stop=True)
                q_sb[p][b] = sbB.tile([128, 512], BF16, tag=f"q{b}", name=f"q{b}")
                nc.vector.tensor_copy(q_sb[p][b], QB)
                KB = pwp.tile([128, 512], F32, tag="pw", name="KB")
                nc.tensor.matmul(out=KB, lhsT=wqkT_s[:, 1, b, :], rhs=srck,
                                 start=True, stop=True)
                k_sb[p][b] = sbB.tile([128, 512], BF16, tag=f"k{b}", name=f"k{b}")
                nc.vector.tensor_copy(k_sb[p][b], KB)

            def emit_vt(p, src):
                """vT for the two branches fed by `src`:
                vt_sb[b] = [128, (sc 4, g 4, ch 32)] bf16, v-only (Z comes
                from the pT-side partition reduce, not from ones columns)."""
                b0 = src            # branches (0,2) from xc, (1,3) from hc
                for b in (b0, b0 + 2):
                    if vt_sb[p][b] is None:
                        vt_sb[p][b] = sbB.tile([128, 512], BF16,
                                               tag=f"vt{b}", name=f"vt{b}")
                src_sb = xc_sb[p] if src == 0 else hc_sb[p]
                sv = src_sb.rearrange("p (s c d) -> p s c d", s=2, c=2)
                for s in range(2):
                    for c in range(2):
                        VT = pwp.tile([128, 256], F32, tag="pw", name="VT")
                        nc.tensor.matmul(out=VT, lhsT=sv[:, s, c, :],
                                         rhs=wvT_s[:, src, :],
                                         start=True, stop=True)
                        sc = s * 2 + c
                        for j, b in enumerate((b0, b0 + 2)):
                            nc.vector.tensor_copy(
                                vt_sb[p][b][:, sc * 128:(sc + 1) * 128],
                                VT[:, j * 128:(j + 1) * 128])

            def emit_cprev(p):
                cprev_sb[p] = sbA.tile([128, 512], F32, tag="cprev", name="cprev")
                nc.sync.dma_start(
                    out=cprev_sb[p],
                    in_=cin[2 * p:2 * p + 2].rearrange("s c q -> c s q"))

            # ---------------- attention iteration pieces ----------------
            def emit_scores_exp(p, b, s):
                """returns pT tile [128, 2048] bf16 = exp(scores^T), layout
                (g, c, q) 4x2x256. One ST bank + one exp per head so PV(g)
                can start as soon as exp(g) lands."""
                kv = k_sb[p][b].rearrange("p (s c d) -> p s c d", s=2, c=2)
                qv = q_sb[p][b].rearrange("p (s q) -> p s q", s=2)
                pT = sbB.tile([128, 2048], BF16, tag="pt", name="pT")
                for h in range(2):
                    ST = stp.tile([128, 1024], F32, tag="st", name="ST")
                    for gg in range(2):
                        g = 2 * h + gg
                        if (h, gg) != (0, 0):
                            dsl = ST[0:32, 512:544] if gg == 1 else ST[0:32, 0:32]
                            nc.tensor.matmul(out=dsl, lhsT=ones_s,
                                             rhs=ones_s, start=True, stop=True,
                                             skip_group_check=True)
                        for c in range(2):
                            nc.tensor.matmul(
                                out=ST[:, gg * 512 + c * 256:gg * 512 + c * 256 + 256],
                                lhsT=kv[32 * g:32 * g + 32, s, c, :],
                                rhs=qv[32 * g:32 * g + 32, s, :],
                                start=True, stop=True, skip_group_check=True,
                                tile_position=(32 * g, 0))
                    nc.scalar.activation(out=pT[:, h * 1024:(h + 1) * 1024],
                                         in_=ST, func=AF.Exp)
                return pT

            def emit_zpath(pT):
                """1/Z from pT alone (runs concurrently with the PV matmuls):
                c-fold on DVE (bf16 fast mode), per-head partition reduce on
                Pool, row assembly on Pool, one DVE reciprocal. Returns rz
                [128, 256] f32 with rows 32g:32g+32 = 1/z_g."""
                pc = epool.tile([128, 1024], BF16, tag="pc", name="pc")
                pv4 = pT.rearrange("p (g c q) -> p g c q", g=4, c=2)
                for g in range(4):
                    nc.vector.tensor_add(pc[:, g * 256:(g + 1) * 256],
                                         pv4[:, g, 0, :], pv4[:, g, 1, :])
                zr = epool.tile([128, 1024], F32, tag="zr", name="zr")
                for g in range(4):
                    nc.gpsimd.partition_all_reduce(
                        zr[:, g * 256:(g + 1) * 256],
                        pc[:, g * 256:(g + 1) * 256],
                        channels=128, reduce_op=RADD)
                zasm = epool.tile([128, 256], F32, tag="zasm", name="zasm")
                for g in range(4):
                    src = zr[32 * g:32 * g + 32, g * 256:(g + 1) * 256]
                    dst = zasm[32 * g:32 * g + 32, :]
                    if g < 2:
                        nc.gpsimd.tensor_copy(dst, src)
                    elif g == 2:
                        nc.scalar.activation(out=dst, in_=src, func=AF.Copy)
                    else:
                        nc.vector.tensor_copy(dst, src)
                rz = epool.tile([128, 256], F32, tag="rz", name="rz")
                nc.vector.reciprocal_approx_fast(out=rz, in_=zasm)
                return rz

            def emit_pvz(p, b, s, pT):
                # A lands PSUM-aligned: head g at partitions 32g:32g+32 of a
                # single [128, 256] tile (half a PSUM bank), M=32 per matmul.
                AZ = azp.tile([128, 256], F32, tag="az", name="AZ")
                for g in range(4):
                    for c in range(2):
                        sc = s * 2 + c
                        nc.tensor.matmul(
                            out=AZ[32 * g:32 * g + 32, :],
                            lhsT=vt_sb[p][b][:, sc * 128 + 32 * g:sc * 128 + 32 * g + 32],
                            rhs=pT[:, g * 512 + c * 256:g * 512 + c * 256 + 256],
                            start=(c == 0), stop=(c == 1), skip_group_check=True,
                            tile_position=(0, 32 * g))
                return AZ

            def emit_norm(p, b, s, AZ, rz):
                """a_all[:, slot] = AZ * rz — a single DVE multiply (AZ is
                already partition-aligned, rz precomputed by the z-path)."""
                slot = b * 2 + s
                if a_all[p] is None:
                    a_all[p] = sbA.tile([128, 2048], F32R, tag="aall", name="a_all")
                nc.vector.tensor_mul(
                    a_all[p][:, slot * 256:(slot + 1) * 256], AZ, rz)

            # ---------------- gates / state / output ----------------
            def emit_gate(p, gi):
                G = pwp.tile([128, 512], F32, tag="pw", name="G")
                av = a_all[p].rearrange("p (b s q) -> p b (s q)", b=4, s=2)
                for b in range(4):
                    nc.tensor.matmul(out=G, lhsT=wtokT_s[:, gi, b, :],
                                     rhs=av[:, b, :],
                                     start=(b == 0), stop=False)
                nc.tensor.matmul(out=G, lhsT=wskipT_s[:, gi, 0, :],
                                 rhs=xc_sb[p], start=False, stop=False)
                nc.tensor.matmul(out=G, lhsT=wskipT_s[:, gi, 1, :],
                                 rhs=hc_sb[p], start=False, stop=True)
                gate_sb[p][gi] = sbA.tile([128, 512], F32, tag=f"gate{gi}",
                                          name=f"gate{gi}")
                # all gates via Tanh: sigmoid(y) = (1+tanh(y/2))/2; the /2 is
                # in GSCALE + host-halved biases, the (1+t)/2 in the update.
                nc.scalar.activation(out=gate_sb[p][gi], in_=G, func=AF.Tanh,
                                     scale=GSCALE[gi], bias=btok_s[:, gi:gi + 1])

            def emit_update_out(p, tail=False):
                ti, tf, tg, to = gate_sb[p]
                # c2 = 2c = (1+tf)*c_prev + (1+ti)*g  (stt is DVE-only;
                # mid-schedule the plain add goes to Pool, in the tail to DVE)
                s1 = sbA.tile([128, 512], F32, tag="fc", name="s1")
                nc.vector.scalar_tensor_tensor(
                    out=s1, in0=tf, scalar=1.0, in1=cprev_sb[p],
                    op0=ALU.add, op1=ALU.mult)
                s2 = sbA.tile([128, 512], F32, tag="ig", name="s2")
                nc.vector.scalar_tensor_tensor(
                    out=s2, in0=ti, scalar=1.0, in1=tg,
                    op0=ALU.add, op1=ALU.mult)
                c2 = sbA.tile([128, 512], F32, tag="c", name="c2")
                (nc.vector if tail else nc.gpsimd).tensor_add(c2, s1, s2)
                # tanh(c) = tanh(c2/2); h2 = 2h = (1+to)*tanh(c); W_out is
                # pre-halved on the host to absorb the remaining 1/2.
                tcs = sbA.tile([128, 512], F32, tag="tc", name="tcs")
                nc.scalar.activation(out=tcs, in_=c2, func=AF.Tanh, scale=0.5)
                hs = sbA.tile([128, 512], F32R, tag="h", name="hs")
                nc.vector.scalar_tensor_tensor(
                    out=hs, in0=to, scalar=1.0, in1=tcs,
                    op0=ALU.add, op1=ALU.mult)
                OUT = pwp.tile([128, 512], F32, tag="pw", name="OUT")
                nc.tensor.matmul(out=OUT, lhsT=woutT_s, rhs=hs,
                                 start=True, stop=True)
                osb = sbA.tile([128, 512], F32, tag="out", name="osb")
                nc.scalar.activation(out=osb, in_=OUT, func=AF.Identity,
                                     bias=bout_s[:, 0:1])
                nc.sync.dma_start(
                    out=yout[2 * p:2 * p + 2].rearrange("s c q -> c s q"),
                    in_=osb.rearrange("p (s q) -> p s q", s=2))

            # ---------------- emission schedule ----------------
            # prologue: pass-0 essentials up to branch 3 (pure hc)
            xt_pad0r, h_pad0r = emit_input_pads(0)
            emit_conv(0, 1, h_pad0r)     # hc pass0
            emit_qk(0, 3)
            emit_vt(0, 1)                # vT for b1, b3 (hc source)
            pads1 = [None]

            def filler(i):
                if i == 0:
                    emit_conv(0, 0, xt_pad0r)         # xc pass0
                elif i == 1:
                    emit_qk(0, 1)
                    emit_qk(0, 2)
                elif i == 2:
                    emit_qk(0, 0)
                    emit_vt(0, 0)
                    emit_cprev(0)
                elif i == 3:
                    pads1[0] = emit_input_pads(1)
                elif i == 4:
                    emit_conv(1, 1, pads1[0][1])      # hc pass1
                elif i == 5:
                    emit_conv(1, 0, pads1[0][0])      # xc pass1
                elif i == 6:
                    emit_qk(1, 3)
                    emit_vt(1, 1)
                elif i == 7:
                    emit_qk(1, 1)
                    emit_qk(1, 2)
                elif i == 8:
                    emit_qk(1, 0)
                    emit_vt(1, 0)
                    emit_cprev(1)
                elif i in (9, 10, 11, 12):
                    emit_gate(0, i - 9)
                elif i == 13:
                    emit_update_out(0)

            iters = [(p, b, s) for p in (0, 1) for b in BORDER for s in (0, 1)]
            prev = None
            for i, (p, b, s) in enumerate(iters):
                pT = emit_scores_exp(p, b, s)
                rz = emit_zpath(pT)
                if prev is not None:
                    pp, pb, ps, ppT, prz = prev
                    AZ = emit_pvz(pp, pb, ps, ppT)
                    emit_norm(pp, pb, ps, AZ, prz)
                prev = (p, b, s, pT, rz)
                filler(i)
            pp, pb, ps, ppT, prz = prev
            AZ = emit_pvz(pp, pb, ps, ppT)
            emit_norm(pp, pb, ps, AZ, prz)
            # tail: f and g feed c2 first; o is only needed after tanh(c)
            for gi in (1, 2, 0, 3):
                emit_gate(1, gi)
            emit_update_out(1, tail=True)

    nc.compile()
    return nc


def _prep_shared(inputs):
    f = np.float32
    c = np.ascontiguousarray
    W_cx, W_ch = np.asarray(inputs["W_cx"], f), np.asarray(inputs["W_ch"], f)
    W_q, W_k, W_v = (np.asarray(inputs[k], f) for k in ("W_q", "W_k", "W_v"))
    W_tok, W_skip = np.asarray(inputs["W_tok"], f), np.asarray(inputs["W_skip"], f)

    w0 = np.zeros((128, W0_END), f)
    w0[0:64, W0_WIN:W0_WIN + 128] = np.asarray(inputs["W_in"], f).T
    w0[:, W0_BIN] = np.asarray(inputs["b_in"], f)
    w0[:, W0_ONES:W0_ONES + 32] = 1.0

    # [i, src, tap, o]
    w1 = np.stack([W_cx.transpose(1, 2, 3, 0).reshape(128, 9, 128),
                   W_ch.transpose(1, 2, 3, 0).reshape(128, 9, 128)],
                  axis=1).reshape(128, W1_END)

    w2 = np.zeros((128, W2_END), f)
    # [c, (q|k), b, a]
    w2[:, W2_QK:W2_QK + 1024] = np.stack(
        [W_q.transpose(2, 0, 1), W_k.transpose(2, 0, 1)], axis=1
    ).reshape(128, 1024)
    # [c, srcpair, a-pair]: xc feeds branches (0,2), hc feeds (1,3)
    w2[:, W2_V:W2_V + 512] = np.stack([
        np.concatenate([W_v[0].T, W_v[2].T], axis=1),
        np.concatenate([W_v[1].T, W_v[3].T], axis=1)], axis=1).reshape(128, 512)

    w3 = np.zeros((128, W3_END), f)
    # [a, gate, branch, r]
    w3[:, W3_TOK:W3_TOK + 2048] = W_tok.transpose(3, 0, 1, 2).reshape(128, 2048)
    # [c, gate, src, r]
    w3[:, W3_SKIP:W3_SKIP + 1024] = W_skip.transpose(3, 0, 1, 2).reshape(128, 1024)
    # W_out pre-halved: h = (1+to)*tanh(c)/2 and the /2 lives here
    w3[:, W3_OUT:W3_OUT + 128] = 0.5 * np.asarray(inputs["W_out"], f).T
    # tanh-form biases: gates i,f,o take tanh(y/2 + b/2)
    btok = np.asarray(inputs["b_tok"], f).T  # [R, 4]
    w3[:, W3_BTOK:W3_BTOK + 4] = btok * np.array([0.5, 0.5, 1.0, 0.5], f)
    w3[:, W3_BOUT] = np.asarray(inputs["b_out"], f)

    return {"w0": c(w0), "w1": c(w1), "w2": c(w2), "w3": c(w3)}


def kernel(**inputs):
    from concourse.bass_utils import run_bass_kernel_spmd
    if "nc" not in _CACHE:
        _CACHE["nc"] = _build_program()
    nc = _CACHE["nc"]
    f = np.float32
    x = np.asarray(inputs["x"], f).reshape(N, I, HW)
    hp = np.asarray(inputs["h_prev"], f).reshape(N, R, HW)
    cp = np.asarray(inputs["c_prev"], f).reshape(N, R, HW)
    shared = _prep_shared(inputs)
    in_maps = []
    for ci in range(NCORES):
        sl = slice(S * ci, S * ci + S)
        m = dict(shared)
        m["xin"] = np.ascontiguousarray(x[sl])
        m["hin"] = np.ascontiguousarray(hp[sl])
        m["cin"] = np.ascontiguousarray(cp[sl])
        in_maps.append(m)
    res = run_bass_kernel_spmd(nc, in_maps, core_ids=list(range(NCORES)))
    y = np.concatenate([r["yout"].reshape(S, R, H, W) for r in res.results],
                       axis=0)
    return y.astype(np.float32)


# revision 56
# speedup vs baseline: 1.2807x; 1.0006x over previous
# Trainium2 Bass kernel for nn_CvtLstm: ConvLSTM cell with 4-branch,
# 4-head spatial attention. Data-parallel over batch N=32 across 8
# NeuronCores (4 samples per core); weights replicated to every core.
#
# Per-core layout: channels on partitions, flattened 16x16 spatial (256)
# on the free dim. conv3x3 = 9 shifted matmuls reading a zero-padded
# [128, 2, 18, 18] tile. Attention scores are computed directly in the
# transposed [d, q] layout (lhsT = per-head k rows, K=32 row-partial
# matmuls); exp on the ACT engine with no max subtraction (scores lie in
# [-9, 8]), bf16 q/k/v/pT (PE speed is identical, halves SBUF).
#
# Softmax normalization (the key restructure vs the first version):
# - The denominator Z never touches PSUM: it is computed from pT (exp
#   output, SBUF) by a c-fold add on DVE (bf16 fast mode) + one Pool
#   partition_all_reduce per head, then a per-head row assembly split
#   across Pool/ACT/DVE and a single DVE reciprocal.
# - PV matmuls are v-only (M=32) with head g written at PSUM partitions
#   32g (tile_position), so A emerges in the final [128, 256] layout and
#   normalization is ONE DVE multiply (PSUM x SBUF -> a_all slot).
# - Zero per-iteration DMAs anywhere (the single HWDGE device costs
#   625ns/descriptor + 900ns sem latency; the old restack DMAs were 80us
#   of queue serialization).
#
# Other engine-budget decisions (from TimelineSim trace analysis):
# - All gates use Tanh (sigmoid(x) = (1+tanh(x/2))/2) so one act table
#   set {exp, tanh, copy} covers the kernel; exp and sigmoid never share
#   a set and each switch would reload tables for 1.28us.
# - LSTM update is fused scalar_tensor_tensor ops: (t+1)*x computes
#   2*gate*x; the 0.5s are folded into tanh scale= and host-halved
#   b_tok/W_out. The final-pass tail orders gates (f,g,i,o) and runs the
#   c2 add on DVE to shorten the critical chain.
# - Zero-padding via Pool memsets, not DMAs. Weights ship in 4 packed
#   DMAs ordered by first use (winT -> conv -> q/k/v -> gates/out).
#
# Hardware constraint honored throughout: two row-partial matmuls at
# different row groups back-to-back fault the device; full-row dummy
# separators are inserted at group changes (see baseline notes).

import numpy as np

N, I, H, W = 32, 64, 16, 16
R, CM, A, HEADS, HC = 128, 128, 128, 4, 32
HW = H * W           # 256
S = 4                # samples per core
NCORES = 8

_CACHE = {}

# packed weight column offsets (f32 words per partition)
W0_WIN, W0_BIN, W0_ONES, W0_END = 0, 128, 129, 161
W1_CONV, W1_END = 0, 2304
W2_QK, W2_V, W2_END = 0, 1024, 1536
W3_TOK, W3_SKIP, W3_OUT, W3_BTOK, W3_BOUT, W3_END = 0, 2048, 3072, 3200, 3204, 3205


def _build_program():
    import contextlib
    import concourse.bacc as bacc
    import concourse.mybir as mybir
    import concourse.tile as tile

    import concourse.bass_isa as bass_isa

    F32 = mybir.dt.float32
    F32R = mybir.dt.float32r
    BF16 = mybir.dt.bfloat16
    AF = mybir.ActivationFunctionType
    ALU = mybir.AluOpType
    RADD = bass_isa.ReduceOp.add

    nc = bacc.Bacc("TRN2", target_bir_lowering=False, debug=False)

    def dram(name, shape, kind="ExternalInput"):
        return nc.dram_tensor(name, list(shape), F32, kind=kind).ap()

    xin = dram("xin", [S, I, HW])
    hin = dram("hin", [S, R, HW])
    cin = dram("cin", [S, R, HW])
    w0d = dram("w0", [128, W0_END])
    w1d = dram("w1", [128, W1_END])
    w2d = dram("w2", [128, W2_END])
    w3d = dram("w3", [128, W3_END])
    yout = dram("yout", [S, R, HW], kind="ExternalOutput")

    QSRC = [0, 0, 1, 1]   # q source per branch: 0=xc, 1=hc
    KSRC = [0, 1, 0, 1]   # k/v source per branch
    BORDER = [3, 1, 2, 0]  # per-pass branch order (b3 = pure hc, earliest)
    GSCALE = [0.5, 0.5, 1.0, 0.5]  # tanh input scale per gate (i,f,g,o)

    with tile.TileContext(nc) as tc:
        with contextlib.ExitStack() as ctx:
            wpool = ctx.enter_context(tc.tile_pool(name="wts", bufs=1))
            sbA = ctx.enter_context(tc.tile_pool(name="sbA", bufs=2))
            sbB = ctx.enter_context(tc.tile_pool(name="sbB", bufs=2))
            epool = ctx.enter_context(tc.tile_pool(name="ep", bufs=4))
            stp = ctx.enter_context(tc.tile_pool(name="st", bufs=2, space="PSUM"))
            azp = ctx.enter_context(tc.tile_pool(name="az", bufs=2, space="PSUM"))
            pwp = ctx.enter_context(tc.tile_pool(name="pw", bufs=2, space="PSUM"))

            # ---------------- input DMAs for pass 0 (issued first so the
            # XT matmul can start while the big weight DMAs stream) -----
            h_pad0 = sbA.tile([128, 648], F32R, tag="hpad", name="hpad")
            xt_pad0 = sbA.tile([128, 648], F32R, tag="xtpad", name="xtpad")
            x2_0 = sbA.tile([64, 2, 256], F32R, tag="x2", name="x2")
            nc.gpsimd.memset(h_pad0.bitcast(F32), 0.0)
            nc.gpsimd.memset(xt_pad0.bitcast(F32), 0.0)
            hv0 = h_pad0.rearrange("p (s y x) -> p s y x", s=2, y=18, x=18)
            for s in range(2):
                nc.sync.dma_start(
                    out=hv0[:, s, 1:17, 1:17],
                    in_=hin[s].rearrange("c (h w) -> c h w", h=16).bitcast(F32R))
            nc.sync.dma_start(
                out=x2_0, in_=xin[0:2].rearrange("s c q -> c s q").bitcast(F32R))

            # ---------------- weights to SBUF (4 packed DMAs) ----------
            def wload(name, src, w):
                t = wpool.tile([128, w], F32R, tag=name, name=name)
                nc.sync.dma_start(out=t, in_=src.bitcast(F32R))
                return t

            w0_s = wload("w0", w0d, W0_END)
            w1_s = wload("w1", w1d, W1_END)
            w2_s = wload("w2", w2d, W2_END)
            w3_s = wload("w3", w3d, W3_END)

            winT_s = w0_s[0:64, W0_WIN:W0_WIN + 128]
            b_in_s = w0_s[:, W0_BIN:W0_BIN + 1].bitcast(F32)
            ones_s = w0_s[:, W0_ONES:W0_ONES + 32]
            wconvT_s = w1_s[:, W1_CONV:W1_CONV + 2304].rearrange(
                "p (s t o) -> p s t o", s=2, t=9)
            wqkT_s = w2_s[:, W2_QK:W2_QK + 1024].rearrange(
                "p (k b o) -> p k b o", k=2, b=4)
            wvT_s = w2_s[:, W2_V:W2_V + 512].rearrange("p (s q) -> p s q", s=2)
            wtokT_s = w3_s[:, W3_TOK:W3_TOK + 2048].rearrange(
                "p (g b o) -> p g b o", g=4, b=4)
            wskipT_s = w3_s[:, W3_SKIP:W3_SKIP + 1024].rearrange(
                "p (g s o) -> p g s o", g=4, s=2)
            woutT_s = w3_s[:, W3_OUT:W3_OUT + 128]
            btok_s = w3_s[:, W3_BTOK:W3_BTOK + 4].bitcast(F32)
            bout_s = w3_s[:, W3_BOUT:W3_BOUT + 1].bitcast(F32)

            # ---------------- per-pass state ----------------
            xc_sb = [None, None]
            hc_sb = [None, None]
            q_sb = [[None] * 4, [None] * 4]
            k_sb = [[None] * 4, [None] * 4]
            vt_sb = [[None] * 4, [None] * 4]
            a_all = [None, None]
            cprev_sb = [None, None]
            gate_sb = [[None] * 4, [None] * 4]

            def emit_input_pads(p):
                """zero-padded xt/h tiles + x2 + XT matmul + tanh.
                For p=0 the pads/DMAs were already emitted above."""
                if p == 0:
                    xt_pad, h_pad, x2 = xt_pad0, h_pad0, x2_0
                else:
                    h_pad = sbA.tile([128, 648], F32R, tag="hpad", name="hpad")
                    xt_pad = sbA.tile([128, 648], F32R, tag="xtpad", name="xtpad")
                    x2 = sbA.tile([64, 2, 256], F32R, tag="x2", name="x2")
                    nc.gpsimd.memset(h_pad.bitcast(F32), 0.0)
                    nc.gpsimd.memset(xt_pad.bitcast(F32), 0.0)
                    hv = h_pad.rearrange("p (s y x) -> p s y x", s=2, y=18, x=18)
                    for s in range(2):
                        nc.sync.dma_start(
                            out=hv[:, s, 1:17, 1:17],
                            in_=hin[2 * p + s].rearrange(
                                "c (h w) -> c h w", h=16).bitcast(F32R))
                    nc.sync.dma_start(
                        out=x2,
                        in_=xin[2 * p:2 * p + 2].rearrange(
                            "s c q -> c s q").bitcast(F32R))
                XT = pwp.tile([128, 512], F32, tag="pw", name="XT")
                nc.tensor.matmul(out=XT, lhsT=winT_s,
                                 rhs=x2.rearrange("p s q -> p (s q)"),
                                 start=True, stop=True)
                xv = xt_pad.rearrange("p (s y x) -> p s y x", s=2, y=18, x=18)
                nc.scalar.activation(
                    out=xv[:, :, 1:17, 1:17],
                    in_=XT.rearrange("p (s h w) -> p s h w", s=2, h=16, w=16),
                    func=AF.Tanh, bias=b_in_s)
                return xt_pad, h_pad

            def emit_conv(p, src, pad):
                """3x3 SAME conv via 9 shifted matmuls; src 0=xc, 1=hc."""
                CP = pwp.tile([128, 512], F32, tag="pw", name="CP")
                pv = pad.rearrange("p (s y x) -> p s y x", s=2, y=18, x=18)
                for t in range(9):
                    ky, kx = divmod(t, 3)
                    nc.tensor.matmul(out=CP, lhsT=wconvT_s[:, src, t, :],
                                     rhs=pv[:, :, ky:ky + 16, kx:kx + 16],
                                     start=(t == 0), stop=(t == 8))
                dst = sbA.tile([128, 512], F32R, tag=("xc" if src == 0 else "hc"),
                               name=("xc" if src == 0 else "hc"))
                nc.vector.tensor_copy(dst, CP)
                if src == 0:
                    xc_sb[p] = dst
                else:
                    hc_sb[p] = dst

            def emit_qk(p, b):
                srcq = xc_sb[p] if QSRC[b] == 0 else hc_sb[p]
                srck = xc_sb[p] if KSRC[b] == 0 else hc_sb[p]
                QB = pwp.tile([128, 512], F32, tag="pw", name="QB")
                nc.tensor.matmul(out=QB, lhsT=wqkT_s[:, 0, b, :], rhs=srcq,
                                 start=True, stop=True)
                q_sb[p][b] = sbB.tile([128, 512], BF16, tag=f"q{b}", name=f"q{b}")
                nc.vector.tensor_copy(q_sb[p][b], QB)
                KB = pwp.tile([128, 512], F32, tag="pw", name="KB")
                nc.tensor.matmul(out=KB, lhsT=wqkT_s[:, 1, b, :], rhs=srck,
                                 start=True, stop=True)
                k_sb[p][b] = sbB.tile([128, 512], BF16, tag=f"k{b}", name=f"k{b}")
                nc.vector.tensor_copy(k_sb[p][b], KB)

            def emit_vt(p, src):
                """vT for the two branches fed by `src`:
                vt_sb[b] = [128, (sc 4, g 4, ch 32)] bf16, v-only (Z comes
                from the pT-side partition reduce, not from ones columns)."""
                b0 = src            # branches (0,2) from xc, (1,3) from hc
                for b in (b0, b0 + 2):
                    if vt_sb[p][b] is None:
                        vt_sb[p][b] = sbB.tile([128, 512], BF16,
                                               tag=f"vt{b}", name=f"vt{b}")
                src_sb = xc_sb[p] if src == 0 else hc_sb[p]
                sv = src_sb.rearrange("p (s c d) -> p s c d", s=2, c=2)
                for s in range(2):
                    for c in range(2):
                        VT = pwp.tile([128, 256], F32, tag="pw", name="VT")
                        nc.tensor.matmul(out=VT, lhsT=sv[:, s, c, :],
                                         rhs=wvT_s[:, src, :],
                                         start=True, stop=True)
                        sc = s * 2 + c
                        for j, b in enumerate((b0, b0 + 2)):
                            nc.vector.tensor_copy(
                                vt_sb[p][b][:, sc * 128:(sc + 1) * 128],
                                VT[:, j * 128:(j + 1) * 128])

            def emit_cprev(p):
                cprev_sb[p] = sbA.tile([128, 512], F32, tag="cprev", name="cprev")
                nc.sync.dma_start(
                    out=cprev_sb[p],
                    in_=cin[2 * p:2 * p + 2].rearrange("s c q -> c s q"))

            # ---------------- attention iteration pieces ----------------
            def emit_scores_exp(p, b, s):
                """returns pT tile [128, 2048] bf16 = exp(scores^T), layout
                (g, c, q) 4x2x256. One ST bank + one exp per head so PV(g)
                can start as soon as exp(g) lands."""
                kv = k_sb[p][b].rearrange("p (s c d) -> p s c d", s=2, c=2)
                qv = q_sb[p][b].rearrange("p (s q) -> p s q", s=2)
                pT = sbB.tile([128, 2048], BF16, tag="pt", name="pT")
                for h in range(2):
                    ST = stp.tile([128, 1024], F32, tag="st", name="ST")
                    for gg in range(2):
                        g = 2 * h + gg
                        if (h, gg) != (0, 0):
                            dsl = ST[0:32, 512:544] if gg == 1 else ST[0:32, 0:32]
                            nc.tensor.matmul(out=dsl, lhsT=ones_s,
                                             rhs=ones_s, start=True, stop=True,
                                             skip_group_check=True)
                        for c in range(2):
                            nc.tensor.matmul(
                                out=ST[:, gg * 512 + c * 256:gg * 512 + c * 256 + 256],
                                lhsT=kv[32 * g:32 * g + 32, s, c, :],
                                rhs=qv[32 * g:32 * g + 32, s, :],
                                start=True, stop=True, skip_group_check=True,
                                tile_position=(32 * g, 0))
                    nc.scalar.activation(out=pT[:, h * 1024:(h + 1) * 1024],
                                         in_=ST, func=AF.Exp)
                return pT

            def emit_zpath(pT):
                """1/Z from pT alone (runs concurrently with the PV matmuls):
                c-fold on DVE (bf16 fast mode), per-head partition reduce on
                Pool, row assembly on Pool, one DVE reciprocal. Returns rz
                [128, 256] f32 with rows 32g:32g+32 = 1/z_g."""
                pc = epool.tile([128, 1024], BF16, tag="pc", name="pc")
                pv4 = pT.rearrange("p (g c q) -> p g c q", g=4, c=2)
                for g in range(4):
                    nc.vector.tensor_add(pc[:, g * 256:(g + 1) * 256],
                                         pv4[:, g, 0, :], pv4[:, g, 1, :])
                zr = epool.tile([128, 1024], F32, tag="zr", name="zr")
                for g in range(4):
                    nc.gpsimd.partition_all_reduce(
                        zr[:, g * 256:(g + 1) * 256],
                        pc[:, g * 256:(g + 1) * 256],
                        channels=128, reduce_op=RADD)
                zasm = epool.tile([128, 256], F32, tag="zasm", name="zasm")
                for g in range(4):
                    src = zr[32 * g:32 * g + 32, g * 256:(g + 1) * 256]
                    dst = zasm[32 * g:32 * g + 32, :]
                    if g < 2:
                        nc.gpsimd.tensor_copy(dst, src)
                    elif g == 2:
                        nc.scalar.activation(out=dst, in_=src, func=AF.Copy)
                    else:
                        nc.vector.tensor_copy(dst, src)
                rz = epool.tile([128, 256], F32, tag="rz", name="rz")
                nc.vector.reciprocal_approx_fast(out=rz, in_=zasm)
                return rz

            def emit_pvz(p, b, s, pT):
                # A lands PSUM-aligned: head g at partitions 32g:32g+32 of a
                # single [128, 256] tile (half a PSUM bank), M=32 per matmul.
                AZ = azp.tile([128, 256], F32, tag="az", name="AZ")
                for g in range(4):
                    for c in range(2):
                        sc = s * 2 + c
                        nc.tensor.matmul(
                            out=AZ[32 * g:32 * g + 32, :],
                            lhsT=vt_sb[p][b][:, sc * 128 + 32 * g:sc * 128 + 32 * g + 32],
                            rhs=pT[:, g * 512 + c * 256:g * 512 + c * 256 + 256],
                            start=(c == 0), stop=(c == 1), skip_group_check=True,
                            tile_position=(0, 32 * g))
                return AZ

            def emit_norm(p, b, s, AZ, rz):
                """a_all[:, slot] = AZ * rz — a single DVE multiply (AZ is
                already partition-aligned, rz precomputed by the z-path)."""
                slot = b * 2 + s
                if a_all[p] is None:
                    a_all[p] = sbA.tile([128, 2048], F32R, tag="aall", name="a_all")
                nc.vector.tensor_mul(
                    a_all[p][:, slot * 256:(slot + 1) * 256], AZ, rz)

            # ---------------- gates / state / output ----------------
            def emit_gate(p, gi):
                G = pwp.tile([128, 512], F32, tag="pw", name="G")
                av = a_all[p].rearrange("p (b s q) -> p b (s q)", b=4, s=2)
                for b in range(4):
                    nc.tensor.matmul(out=G, lhsT=wtokT_s[:, gi, b, :],
                                     rhs=av[:, b, :],
                                     start=(b == 0), stop=False)
                nc.tensor.matmul(out=G, lhsT=wskipT_s[:, gi, 0, :],
                                 rhs=xc_sb[p], start=False, stop=False)
                nc.tensor.matmul(out=G, lhsT=wskipT_s[:, gi, 1, :],
                                 rhs=hc_sb[p], start=False, stop=True)
                gate_sb[p][gi] = sbA.tile([128, 512], F32, tag=f"gate{gi}",
                                          name=f"gate{gi}")
                # all gates via Tanh: sigmoid(y) = (1+tanh(y/2))/2; the /2 is
                # in GSCALE + host-halved biases, the (1+t)/2 in the update.
                nc.scalar.activation(out=gate_sb[p][gi], in_=G, func=AF.Tanh,
                                     scale=GSCALE[gi], bias=btok_s[:, gi:gi + 1])

            def emit_update_out(p, tail=False):
                ti, tf, tg, to = gate_sb[p]
                # c2 = 2c = (1+tf)*c_prev + (1+ti)*g  (stt is DVE-only;
                # mid-schedule the plain add goes to Pool, in the tail to DVE)
                s1 = sbA.tile([128, 512], F32, tag="fc", name="s1")
                nc.vector.scalar_tensor_tensor(
                    out=s1, in0=tf, scalar=1.0, in1=cprev_sb[p],
                    op0=ALU.add, op1=ALU.mult)
                s2 = sbA.tile([128, 512], F32, tag="ig", name="s2")
                nc.vector.scalar_tensor_tensor(
                    out=s2, in0=ti, scalar=1.0, in1=tg,
                    op0=ALU.add, op1=ALU.mult)
                c2 = sbA.tile([128, 512], F32, tag="c", name="c2")
                (nc.vector if tail else nc.gpsimd).tensor_add(c2, s1, s2)
                # tanh(c) = tanh(c2/2); h2 = 2h = (1+to)*tanh(c); W_out is
                # pre-halved on the host to absorb the remaining 1/2.
                tcs = sbA.tile([128, 512], F32, tag="tc", name="tcs")
                nc.scalar.activation(out=tcs, in_=c2, func=AF.Tanh, scale=0.5)
                hs = sbA.tile([128, 512], F32R, tag="h", name="hs")
                nc.vector.scalar_tensor_tensor(
                    out=hs, in0=to, scalar=1.0, in1=tcs,
                    op0=ALU.add, op1=ALU.mult)
                OUT = pwp.tile([128, 512], F32, tag="pw", name="OUT")
                nc.tensor.matmul(out=OUT, lhsT=woutT_s, rhs=hs,
                                 start=True, stop=True)
                osb = sbA.tile([128, 512], F32, tag="out", name="osb")
                nc.scalar.activation(out=osb, in_=OUT, func=AF.Identity,
                                     bias=bout_s[:, 0:1])
                nc.sync.dma_start(
                    out=yout[2 * p:2 * p + 2].rearrange("s c q -> c s q"),
                    in_=osb.rearrange("p (s q) -> p s q", s=2))

            # ---------------- emission schedule ----------------
            # prologue: pass-0 essentials up to branch 3 (pure hc)
            xt_pad0r, h_pad0r = emit_input_pads(0)
            emit_conv(0, 1, h_pad0r)     # hc pass0
            emit_qk(0, 3)
            emit_vt(0, 1)                # vT for b1, b3 (hc source)
            pads1 = [None]

            def filler(i):
                if i == 0:
                    emit_conv(0, 0, xt_pad0r)         # xc pass0
                elif i == 1:
                    emit_qk(0, 1)
                    emit_qk(0, 2)
                elif i == 2:
                    emit_qk(0, 0)
                    emit_vt(0, 0)
                    emit_cprev(0)
                elif i == 3:
                    pads1[0] = emit_input_pads(1)
                elif i == 4:
                    emit_conv(1, 1, pads1[0][1])      # hc pass1
                elif i == 5:
                    emit_conv(1, 0, pads1[0][0])      # xc pass1
                elif i == 6:
                    emit_qk(1, 3)
                    emit_vt(1, 1)
                elif i == 7:
                    emit_qk(1, 1)
                    emit_qk(1, 2)
                elif i == 8:
                    emit_qk(1, 0)
                    emit_vt(1, 0)
                    emit_cprev(1)
                elif i in (9, 10, 11, 12):
                    emit_gate(0, i - 9)
                elif i == 13:
                    emit_update_out(0)

            iters = [(p, b, s) for p in (0, 1) for b in BORDER for s in (0, 1)]
            prev = None
            for i, (p, b, s) in enumerate(iters):
                pT = emit_scores_exp(p, b, s)
                rz = emit_zpath(pT)
                if prev is not None:
                    pp, pb, ps, ppT, prz = prev
                    AZ = emit_pvz(pp, pb, ps, ppT)
                    emit_norm(pp, pb, ps, AZ, prz)
                prev = (p, b, s, pT, rz)
                filler(i)
            pp, pb, ps, ppT, prz = prev
            AZ = emit_pvz(pp, pb, ps, ppT)
            emit_norm(pp, pb, ps, AZ, prz)
            # tail: f and g feed c2 first; o is only needed after tanh(c)
            for gi in (1, 2, 0, 3):
                emit_gate(1, gi)
            emit_update_out(1, tail=True)

    nc.compile()
    return nc


def _prep_shared(inputs):
    f = np.float32
    c = np.ascontiguousarray
    W_cx, W_ch = np.asarray(inputs["W_cx"], f), np.asarray(inputs["W_ch"], f)
    W_q, W_k, W_v = (np.asarray(inputs[k], f) for k in ("W_q", "W_k", "W_v"))
    W_tok, W_skip = np.asarray(inputs["W_tok"], f), np.asarray(inputs["W_skip"], f)

    w0 = np.zeros((128, W0_END), f)
    w0[0:64, W0_WIN:W0_WIN + 128] = np.asarray(inputs["W_in"], f).T
    w0[:, W0_BIN] = np.asarray(inputs["b_in"], f)
    w0[:, W0_ONES:W0_ONES + 32] = 1.0

    # [i, src, tap, o]
    w1 = np.stack([W_cx.transpose(1, 2, 3, 0).reshape(128, 9, 128),
                   W_ch.transpose(1, 2, 3, 0).reshape(128, 9, 128)],
                  axis=1).reshape(128, W1_END)

    w2 = np.zeros((128, W2_END), f)
    # [c, (q|k), b, a]
    w2[:, W2_QK:W2_QK + 1024] = np.stack(
        [W_q.transpose(2, 0, 1), W_k.transpose(2, 0, 1)], axis=1
    ).reshape(128, 1024)
    # [c, srcpair, a-pair]: xc feeds branches (0,2), hc feeds (1,3)
    w2[:, W2_V:W2_V + 512] = np.stack([
        np.concatenate([W_v[0].T, W_v[2].T], axis=1),
        np.concatenate([W_v[1].T, W_v[3].T], axis=1)], axis=1).reshape(128, 512)

    w3 = np.zeros((128, W3_END), f)
    # [a, gate, branch, r]
    w3[:, W3_TOK:W3_TOK + 2048] = W_tok.transpose(3, 0, 1, 2).reshape(128, 2048)
    # [c, gate, src, r]
    w3[:, W3_SKIP:W3_SKIP + 1024] = W_skip.transpose(3, 0, 1, 2).reshape(128, 1024)
    # W_out pre-halved: h = (1+to)*tanh(c)/2 and the /2 lives here
    w3[:, W3_OUT:W3_OUT + 128] = 0.5 * np.asarray(inputs["W_out"], f).T
    # tanh-form biases: gates i,f,o take tanh(y/2 + b/2)
    btok = np.asarray(inputs["b_tok"], f).T  # [R, 4]
    w3[:, W3_BTOK:W3_BTOK + 4] = btok * np.array([0.5, 0.5, 1.0, 0.5], f)
    w3[:, W3_BOUT] = np.asarray(inputs["b_out"], f)

    return {"w0": c(w0), "w1": c(w1), "w2": c(w2), "w3": c(w3)}


def kernel(**inputs):
    from concourse.bass_utils import run_bass_kernel_spmd
    if "nc" not in _CACHE:
        _CACHE["nc"] = _build_program()
    nc = _CACHE["nc"]
    f = np.float32
    x = np.asarray(inputs["x"], f).reshape(N, I, HW)
    hp = np.asarray(inputs["h_prev"], f).reshape(N, R, HW)
    cp = np.asarray(inputs["c_prev"], f).reshape(N, R, HW)
    shared = _prep_shared(inputs)
    in_maps = []
    for ci in range(NCORES):
        sl = slice(S * ci, S * ci + S)
        m = dict(shared)
        m["xin"] = np.ascontiguousarray(x[sl])
        m["hin"] = np.ascontiguousarray(hp[sl])
        m["cin"] = np.ascontiguousarray(cp[sl])
        in_maps.append(m)
    res = run_bass_kernel_spmd(nc, in_maps, core_ids=list(range(NCORES)))
    y = np.concatenate([r["yout"].reshape(S, R, H, W) for r in res.results],
                       axis=0)
    return y.astype(np.float32)
